# revision 1
# baseline (speedup 1.0000x reference)
"""Distributed Trainium2 (8-core) kernel for the GCN+AGNN message-passing model.

Strategy (destination-sharded, window-tiled gathers):
- Nodes are degree-sorted and snake-assigned to 8 cores (12544 slots/core incl
  dummies). Each core computes the input MLP for its shard in channel-major
  layout, derives the packed per-node feature row [hn=dinv*h | xn=h/|h|] (f32,
  512B), and an AllGather replicates the full 100352-row table to every core.
- Edges go to the core owning their destination. Because dma_gather indices
  are int16, sources are split into 4 windows of 32768 table rows. Per window,
  local destinations are sorted by in-count, bucketed 128-wide, padded to the
  bucket max K (pads hit an all-zero in-window row). One dma_gather per
  subgroup fetches [128 dests, K slots, 128ch] tiles; VectorE segmented
  reduces produce GCN aggregate / AGNN softmax numerator+denominator partials
  which dma_scatter_add accumulates into a [12545, 192] f32 DRAM accumulator.
- Epilogue (per 512 dests): add analytic self-loop terms, finish the AGNN
  softmax, PE-transpose to channel-major and run the small GCN/projection/
  classifier matmuls; host inverse-permutes the [2, 12544] per-core outputs.
"""
import os
import sys

for _p in ("/opt/trn_rl_repo", "/root/.axon_site/_ro/trn_rl_repo"):
    if os.path.isdir(_p) and _p not in sys.path:
        sys.path.insert(0, _p)

import numpy as np

NCORE = 8
N = 100000
HID = 64
CH = 128
P = 128
DCORE = 12544
NBUCK = DCORE // P          # 98
TABLE_ROWS = NCORE * DCORE  # 100352
WIN = 32768
NWIN = 4
SMAX = 64
NBMAX = 8
ACC_W = 192                 # f32 accumulator row -> 768B
ACC_ROWS = DCORE + 1
PAD_DEST = 12500
EPS = 1e-12
CHUNK = 4                   # buckets per epilogue chunk (512 dests)
NCHUNK = NBUCK // CHUNK + (1 if NBUCK % CHUNK else 0)   # 25 (24x4 + 1x2)


def _chunks():
    out = []
    b = 0
    while b < NBUCK:
        nb = min(CHUNK, NBUCK - b)
        out.append((b, nb))
        b += nb
    return out


def _wrap_idx(idxs):
    n = len(idxs)
    nc16 = (n + 15) // 16
    w = np.zeros((16, nc16), np.int16)
    w[np.arange(n) % 16, np.arange(n) // 16] = idxs
    return np.tile(w, (8, 1))


def _prep(x, edge_index):
    x = np.asarray(x, np.float32)
    row = np.asarray(edge_index[0], np.int64)
    col = np.asarray(edge_index[1], np.int64)
    deg = np.bincount(col, minlength=N).astype(np.int64) + 1
    dinv = (deg.astype(np.float64) ** -0.5).astype(np.float32)
    sdeg = np.sqrt(deg.astype(np.float32))

    c_in = deg - 1
    order = np.argsort(-c_in, kind="stable")
    pos = np.arange(N)
    r, j = pos // NCORE, pos % NCORE
    core_of_pos = np.where(r % 2 == 0, j, NCORE - 1 - j)
    node_slot = np.empty(N, np.int64)
    slot_node = np.full(TABLE_ROWS, -1, np.int64)
    for k in range(NCORE):
        nodes_k = order[core_of_pos == k]
        slots = k * DCORE + np.arange(len(nodes_k))
        node_slot[nodes_k] = slots
        slot_node[slots] = nodes_k

    ZROW = []
    for w in range(NWIN):
        lo, hi = w * WIN, min((w + 1) * WIN, TABLE_ROWS)
        dum = np.where(slot_node[lo:hi] < 0)[0]
        assert len(dum) > 0, f"window {w} has no dummy slot for a zero row"
        ZROW.append(int(lo + dum[0]))

    src_slot = node_slot[row]
    dst_slot = node_slot[col]
    dst_core = dst_slot // DCORE
    dst_local = dst_slot % DCORE
    src_win = src_slot // WIN

    counts = np.zeros((NCORE, NWIN, DCORE), np.int64)
    for k in range(NCORE):
        m = dst_core == k
        for w in range(NWIN):
            counts[k, w] = np.bincount(dst_local[m & (src_win == w)], minlength=DCORE)

    pi = np.zeros((NCORE, NWIN, DCORE), np.int64)
    csort = np.zeros((NCORE, NWIN, DCORE), np.int64)
    for k in range(NCORE):
        for w in range(NWIN):
            o = np.argsort(-counts[k, w], kind="stable")
            pi[k, w] = o
            csort[k, w] = counts[k, w][o]

    structure = []
    for w in range(NWIN):
        nz = int(max((csort[k, w] > 0).sum() for k in range(NCORE)))
        nb_w = (nz + P - 1) // P
        K_b = [int(csort[:, w, b * P].max()) for b in range(nb_w)]
        subs = []
        b = 0
        while b < nb_w:
            K = K_b[b]
            nb = 1
            while (b + nb < nb_w and K_b[b + nb] == K and nb < NBMAX
                   and (nb + 1) * K <= SMAX):
                nb += 1
            subs.append({"b0": b, "nb": nb, "K": K})
            b += nb
        structure.append(subs)

    cores = []
    for k in range(NCORE):
        m = dst_core == k
        es_k = src_slot[m]
        ed_k = dst_local[m]
        gidx_parts, xidx_parts, sidx_parts, sdeg_parts = [], [], [], []
        padcnt = np.zeros(DCORE, np.int64)
        for w in range(NWIN):
            inv_pi = np.empty(DCORE, np.int64)
            inv_pi[pi[k, w]] = np.arange(DCORE)
            mw = (es_k // WIN) == w
            es, ed = es_k[mw], ed_k[mw]
            rank = inv_pi[ed]
            o = np.lexsort((es, rank))
            es, rank = es[o], rank[o]
            bc = np.bincount(rank, minlength=DCORE)
            jj = np.arange(len(rank)) - np.repeat(
                np.concatenate([[0], np.cumsum(bc)[:-1]]), bc)
            for sub in structure[w]:
                b0, nb, K = sub["b0"], sub["nb"], sub["K"]
                S = nb * K
                g = np.full((S, P), ZROW[w] - w * WIN, np.int64)
                sd = np.zeros((P, S), np.float32)
                sel = (rank >= b0 * P) & (rank < (b0 + nb) * P)
                rr, ee, jx = rank[sel], es[sel], jj[sel]
                bi = rr // P - b0
                d = rr % P
                slot = bi * K + jx
                g[slot, d] = ee - w * WIN
                sd[d, slot] = sdeg[slot_node[ee]]
                gidx_parts.append(_wrap_idx(g.reshape(-1).astype(np.int16)))
                sdeg_parts.append(sd.astype(np.float16))
                q = np.arange(b0 * P, (b0 + nb) * P)
                xd = np.where(q < DCORE, pi[k, w][np.minimum(q, DCORE - 1)], PAD_DEST)
                xidx_parts.append(_wrap_idx(xd.astype(np.int16)))
                sidx_parts.append(_wrap_idx(np.where(q < DCORE, xd, ACC_ROWS - 1).astype(np.int16)))
                cw = counts[k, w][xd[q < DCORE]]
                padcnt[xd[q < DCORE]] += K - cw
        ld = (np.arange(NBUCK)[None, :] * P + np.arange(P)[:, None])
        node_of_ld = slot_node[k * DCORE + ld]
        real = node_of_ld >= 0
        dinvc = np.where(real, dinv[np.maximum(node_of_ld, 0)], 1.0).astype(np.float32)
        padneg = np.where(real, -padcnt[ld].astype(np.float32), 0.0).astype(np.float32)
        xp = np.zeros((DCORE, HID), np.float32)
        sel = slot_node[k * DCORE:(k + 1) * DCORE] >= 0
        xp[sel] = x[slot_node[k * DCORE:(k + 1) * DCORE][sel]]
        cores.append({
            "gidx": np.concatenate(gidx_parts, axis=1),
            "xidx": np.concatenate(xidx_parts, axis=1),
            "sidx": np.concatenate(sidx_parts, axis=1),
            "sdeg": np.concatenate(sdeg_parts, axis=1),
            "dinvc": dinvc, "padneg": padneg, "xpT": np.ascontiguousarray(xp.T),
        })
    meta = {"structure": structure, "ZROW": ZROW, "slot_node": slot_node}
    return cores, meta


def _build(structure, zrows, gw, xw, sw):
    """Build the SPMD Bass program. gw/xw/sw: widths of the flat idx/sdeg arrays."""
    KDBG = os.environ.get("KDBG", "")
    KEDGE = int(os.environ.get("KEDGE", "9999"))
    KSKIP = set(os.environ.get("KSKIP", "").split(","))
    KREPS = int(os.environ.get("KREPS", "1"))
    KR_BUILD = int(os.environ.get("KR_BUILD", "1"))
    KR_COLL = int(os.environ.get("KR_COLL", "1"))
    KR_EDGE = int(os.environ.get("KR_EDGE", "1"))
    KR_EPI = int(os.environ.get("KR_EPI", "1"))
    from concourse import bass, bacc, mybir, tile
    from concourse.masks import make_identity

    f32, f16, i16 = mybir.dt.float32, mybir.dt.float16, mybir.dt.int16
    AX = mybir.AxisListType
    OP = mybir.AluOpType
    AF = mybir.ActivationFunctionType

    nc = bacc.Bacc("TRN2", target_bir_lowering=False, debug=False,
                   enable_asserts=False, num_devices=NCORE,
                   num_swdge_queues=4)

    xpT = nc.dram_tensor("xpT", [HID, DCORE], f32, kind="ExternalInput")
    gidx = nc.dram_tensor("gidx", [P, gw], i16, kind="ExternalInput")
    xidx = nc.dram_tensor("xidx", [P, xw], i16, kind="ExternalInput")
    sidx = nc.dram_tensor("sidx", [P, xw], i16, kind="ExternalInput")
    sdegt = nc.dram_tensor("sdegt", [P, sw], f16, kind="ExternalInput")
    dinvt = nc.dram_tensor("dinvt", [P, NBUCK], f32, kind="ExternalInput")
    padnt = nc.dram_tensor("padnt", [P, NBUCK], f32, kind="ExternalInput")
    wnames = ["w1", "w2", "w3", "wg1", "wg2", "wf", "wx"]
    wts = {n: nc.dram_tensor(n, [HID, HID], f32, kind="ExternalInput") for n in wnames}
    wc0t = nc.dram_tensor("wc0", [HID, 2], f32, kind="ExternalInput")
    wc1t = nc.dram_tensor("wc1", [HID, 2], f32, kind="ExternalInput")
    bnames = ["b1", "b2", "b3", "bg1", "bg2", "bf", "bx"]
    bts = {n: nc.dram_tensor(n, [HID, 1], f32, kind="ExternalInput") for n in bnames}
    bct = nc.dram_tensor("bc", [2, 1], f32, kind="ExternalInput")
    betat = nc.dram_tensor("beta", [1, 1], f32, kind="ExternalInput")
    out = nc.dram_tensor("out", [2, DCORE], f32, kind="ExternalOutput")

    shard = nc.dram_tensor("shard", [DCORE, CH], f32)
    table = nc.dram_tensor("table", [TABLE_ROWS, CH], f32)
    hT_d = nc.dram_tensor("hT_d", [HID, DCORE], f32)
    accum = nc.dram_tensor("accum", [ACC_ROWS, ACC_W], f32)
    if KDBG == "build":
        dbg = nc.dram_tensor("dbg", [P, CH], f32, kind="ExternalOutput")
    if KDBG == "edge":
        dbga = nc.dram_tensor("dbga", [ACC_ROWS, ACC_W], f32, kind="ExternalOutput")

    chunks = _chunks()

    with tile.TileContext(nc) as tc:
        with tc.tile_pool(name="const", bufs=1) as cpool, \
             tc.tile_pool(name="persist", bufs=1) as ppool:

            # ---- constants ----
            wsb = {}
            for n in wnames:
                t = cpool.tile([HID, HID], f32, name=f"w_{n}")
                nc.sync.dma_start(out=t[:], in_=wts[n][:])
                wsb[n] = t
            wc0_sb = cpool.tile([HID, 2], f32)
            nc.sync.dma_start(out=wc0_sb[:], in_=wc0t[:])
            wc1_sb = cpool.tile([HID, 2], f32)
            nc.sync.dma_start(out=wc1_sb[:], in_=wc1t[:])
            bsb = {}
            for n in bnames:
                t = cpool.tile([HID, 1], f32, name=f"b_{n}")
                nc.sync.dma_start(out=t[:], in_=bts[n][:])
                bsb[n] = t
            bc_sb = cpool.tile([2, 1], f32)
            nc.sync.dma_start(out=bc_sb[:], in_=bct[:])
            beta1 = cpool.tile([1, 1], f32)
            nc.sync.dma_start(out=beta1[:], in_=betat[:])
            beta128 = cpool.tile([P, 1], f32)
            nc.gpsimd.partition_broadcast(beta128[:], beta1[:])
            ident = cpool.tile([P, P], f32)
            make_identity(nc, ident[:])
            ones_col = cpool.tile([HID, 1], f32)
            nc.vector.memset(ones_col[:], 1.0)
            ones_row = cpool.tile([1, HID], f32)
            nc.vector.memset(ones_row[:], 1.0)
            zrow_sb = cpool.tile([1, CH], f32)
            nc.vector.memset(zrow_sb[:], 0.0)
            zacc = cpool.tile([P, ACC_W], f32)
            nc.vector.memset(zacc[:], 0.0)
            epsb = cpool.tile([P, 1], f32)
            nc.vector.memset(epsb[:], float(EPS))
            dinv_sb = ppool.tile([P, NBUCK], f32)
            nc.sync.dma_start(out=dinv_sb[:], in_=dinvt[:])
            padn_sb = ppool.tile([P, NBUCK], f32)
            nc.sync.dma_start(out=padn_sb[:], in_=padnt[:])
            h_dm = ppool.tile([P, NBUCK, HID], f32)
            nx2_sb = ppool.tile([P, NBUCK], f32)

            for _rep in range(KREPS):
                # ---- zero the accumulator ----
                for r0 in range(0, ACC_ROWS, P):
                    r1 = min(r0 + P, ACC_ROWS)
                    nc.sync.dma_start(out=accum[r0:r1, :], in_=zacc[:r1 - r0, :])

                # ---- build phase ----
                with tc.tile_pool(name="build", bufs=2) as bpool, \
                     tc.tile_pool(name="bpsum", bufs=2, space="PSUM") as bpsum:
                  for _rb in range(KR_BUILD):
                    for ci, (cb, cnb) in enumerate(chunks):
                        ncol = cnb * P
                        c0 = cb * P
                        xc = bpool.tile([HID, ncol], f32, tag="xc")
                        nc.sync.dma_start(out=xc[:], in_=xpT[:, c0:c0 + ncol])
                        pm = bpsum.tile([HID, ncol], f32, tag="bp1")
                        nc.tensor.matmul(pm[:], wsb["w1"][:], xc[:], start=True, stop=True)
                        hh1 = bpool.tile([HID, ncol], f32, tag="hh1")
                        nc.scalar.activation(hh1[:], pm[:], AF.Relu, bias=bsb["b1"][:, :])
                        pm2 = bpsum.tile([HID, ncol], f32, tag="bp2")
                        nc.tensor.matmul(pm2[:], wsb["w2"][:], hh1[:], start=True, stop=True)
                        hh2 = bpool.tile([HID, ncol], f32, tag="hh2")
                        nc.scalar.activation(hh2[:], pm2[:], AF.Relu, bias=bsb["b2"][:, :])
                        pm3 = bpsum.tile([HID, ncol], f32, tag="bp3")
                        nc.tensor.matmul(pm3[:], wsb["w3"][:], hh2[:], start=True, stop=True)
                        hTc = bpool.tile([HID, ncol], f32, tag="hTc")
                        nc.scalar.activation(hTc[:], pm3[:], AF.Identity, bias=bsb["b3"][:, :])
                        nc.sync.dma_start(out=hT_d[:, c0:c0 + ncol], in_=hTc[:])
                        # transpose to dest-major: in [64, 128] -> out [128, 64]
                        for b in range(cnb):
                            tp = bpsum.tile([P, HID], f32, tag="btp")
                            nc.tensor.transpose(tp[:], hTc[:, b * P:(b + 1) * P],
                                                ident[0:HID, 0:HID])
                            nc.scalar.copy(out=h_dm[:, cb + b, :], in_=tp[:])
                        hd = h_dm[:, cb:cb + cnb, :]
                        sq = bpool.tile([P, cnb, HID], f32, tag="sq")
                        nc.scalar.activation(sq[:], hd, AF.Square)
                        n2 = bpool.tile([P, cnb], f32, tag="n2")
                        nc.vector.tensor_reduce(out=n2[:], in_=sq[:], axis=AX.X, op=OP.add)
                        nrm = bpool.tile([P, cnb], f32, tag="nrm")
                        nc.scalar.activation(nrm[:], n2[:], AF.Sqrt)
                        nrme = bpool.tile([P, cnb], f32, tag="nrme")
                        nc.scalar.activation(nrme[:], nrm[:], AF.Identity, bias=epsb[:, :])
                        rn = bpool.tile([P, cnb], f32, tag="rn")
                        nc.vector.reciprocal(rn[:], nrme[:])
                        nx = bpool.tile([P, cnb], f32, tag="nx")
                        nc.vector.tensor_tensor(out=nx[:], in0=nrm[:], in1=rn[:], op=OP.mult)
                        nc.scalar.activation(nx2_sb[:, cb:cb + cnb], nx[:], AF.Square)
                        st = bpool.tile([P, cnb, CH], f32, tag="st")
                        nc.vector.tensor_tensor(
                            out=st[:, :, 0:HID], in0=hd,
                            in1=dinv_sb[:, cb:cb + cnb].unsqueeze(2).to_broadcast([P, cnb, HID]),
                            op=OP.mult)
                        nc.vector.tensor_tensor(
                            out=st[:, :, HID:CH], in0=hd,
                            in1=rn[:].unsqueeze(2).to_broadcast([P, cnb, HID]),
                            op=OP.mult)
                        nc.sync.dma_start(
                            out=shard[c0:c0 + ncol, :].rearrange("(b p) c -> p b c", p=P),
                            in_=st[:])

                # ---- all-gather the table, then punch the per-window zero rows ----
                for _rc in range(KR_COLL):
                    nc.gpsimd.collective_compute(
                        "AllGather", mybir.AluOpType.bypass,
                        replica_groups=[list(range(NCORE))],
                        ins=[shard[:, :]], outs=[table[:, :]])
                for z in zrows:
                    nc.sync.dma_start(out=table[z:z + 1, :], in_=zrow_sb[:])

                if KDBG == "build":
                    with tc.tile_pool(name="dbgp", bufs=1) as dp:
                        dt_ = dp.tile([P, CH], f32)
                        nc.sync.dma_start(out=dt_[:], in_=table[50000:50000 + P, :])
                        nc.sync.dma_start(out=dbg[:], in_=dt_[:])
                # ---- edge phase ----
                with tc.tile_pool(name="io", bufs=3) as io, \
                     tc.tile_pool(name="edge", bufs=2) as ep:
                  for _re in range(KR_EDGE):
                    g16 = x16 = soff = 0
                    nsub = 0
                    qrr = [0]
                    for w in (range(NWIN) if KDBG != "build" else []):
                        base = w * WIN
                        bend = min((w + 1) * WIN, TABLE_ROWS)
                        for sub in structure[w]:
                            nb, K = sub["nb"], sub["K"]
                            S = nb * K
                            nsub += 1
                            if nsub > KEDGE:
                                g16 += S * 8
                                x16 += nb * 8
                                soff += S
                                continue
                            it = io.tile([P, S * 8], i16, tag="it")
                            nc.sync.dma_start(out=it[:], in_=gidx[:, g16:g16 + S * 8])
                            xt = io.tile([P, nb * 8], i16, tag="xt")
                            nc.sync.dma_start(out=xt[:], in_=xidx[:, x16:x16 + nb * 8])
                            stx = io.tile([P, nb * 8], i16, tag="stx")
                            nc.sync.dma_start(out=stx[:], in_=sidx[:, x16:x16 + nb * 8])
                            sd = io.tile([P, S], f16, tag="sd")
                            nc.sync.dma_start(out=sd[:], in_=sdegt[:, soff:soff + S])
                            g16 += S * 8
                            x16 += nb * 8
                            soff += S

                            gt = ep.tile([P, S, CH], f32, tag="gt")
                            if "gt" in KSKIP:
                                nc.vector.memset(gt[:, :, :], 0.0)
                            else:
                                for j0 in range(0, S, 8):
                                    ns = min(8, S - j0)
                                    nc.gpsimd.dma_gather(
                                        out_ap=gt[:, j0:j0 + ns, :],
                                        in_ap=table[base:bend, :],
                                        idxs_ap=it[:, j0 * 8:(j0 + ns) * 8],
                                        num_idxs=ns * P, num_idxs_reg=ns * P,
                                        elem_size=CH, queue_num=qrr[0] % 3)
                                    qrr[0] += 1
                            xc2 = ep.tile([P, nb, CH], f32, tag="xc2")
                            if "xc" in KSKIP:
                                nc.vector.memset(xc2[:, :, :], 0.0)
                            else:
                                nc.gpsimd.dma_gather(
                                    out_ap=xc2[:, :, :], in_ap=shard[:, :],
                                    idxs_ap=xt[:, :], num_idxs=nb * P, num_idxs_reg=nb * P,
                                    elem_size=CH, queue_num=qrr[0] % 3)
                                qrr[0] += 1

                            gat = gt[:, :, :].rearrange("p (b k) c -> p b k c", b=nb)
                            tmpa = ep.tile([P, S, HID], f16, tag="tmpa")
                            nc.vector.tensor_tensor(
                                out=tmpa[:, :, :].rearrange("p (b k) c -> p b k c", b=nb),
                                in0=gat[:, :, :, HID:CH],
                                in1=xc2[:, :, HID:CH].unsqueeze(2).to_broadcast([P, nb, K, HID]),
                                op=OP.mult)
                            alpha = ep.tile([P, S], f32, tag="alpha")
                            nc.vector.tensor_reduce(out=alpha[:], in_=tmpa[:, :, :],
                                                    axis=AX.X, op=OP.add)
                            e = ep.tile([P, S], f16, tag="e")
                            nc.scalar.activation(e[:], alpha[:], AF.Exp, scale=beta128[:, :])
                            epw = ep.tile([P, S], f16, tag="epw")
                            nc.vector.tensor_tensor(out=epw[:], in0=e[:], in1=sd[:], op=OP.mult)
                            tmpn = ep.tile([P, S, HID], f16, tag="tmpn")
                            nc.vector.tensor_tensor(
                                out=tmpn[:, :, :], in0=gt[:, :, 0:HID],
                                in1=epw[:].unsqueeze(2).to_broadcast([P, S, HID]),
                                op=OP.mult)
                            part = ep.tile([P, nb, ACC_W], f32, tag="part")
                            nc.vector.memset(part[:, :, CH + 1:ACC_W], 0.0)
                            nc.vector.tensor_reduce(
                                out=part[:, :, CH:CH + 1],
                                in_=e[:].rearrange("p (b k) -> p b k", b=nb),
                                axis=AX.X, op=OP.add)
                            nc.vector.tensor_reduce(
                                out=part[:, :, 0:HID],
                                in_=gat[:, :, :, 0:HID].rearrange("p b k c -> p b c k"),
                                axis=AX.X, op=OP.add)
                            nc.vector.tensor_reduce(
                                out=part[:, :, HID:CH],
                                in_=tmpn[:, :, :].rearrange("p (b k) c -> p b c k", b=nb),
                                axis=AX.X, op=OP.add)
                            if "sc" not in KSKIP:
                                nc.gpsimd.dma_scatter_add(
                                    out_ap=accum[:, :], in_ap=part[:, :, :], idxs_ap=stx[:, :],
                                    num_idxs=nb * P, num_idxs_reg=nb * P, elem_size=ACC_W,
                                    queue_num=3)

                if KDBG == "edge":
                    with tc.tile_pool(name="dbgp2", bufs=1) as dp2:
                        for r0 in range(0, ACC_ROWS, P):
                            r1 = min(r0 + P, ACC_ROWS)
                            da = dp2.tile([P, ACC_W], f32, tag="da")
                            nc.sync.dma_start(out=da[:r1 - r0, :], in_=accum[r0:r1, :])
                            nc.sync.dma_start(out=dbga[r0:r1, :], in_=da[:r1 - r0, :])
                # ---- epilogue ----
                with tc.tile_pool(name="epi", bufs=2) as epi, \
                     tc.tile_pool(name="epsum", bufs=3, space="PSUM") as epsum, \
                     tc.tile_pool(name="epsum2", bufs=2, space="PSUM") as epsum2:
                  for _rp in range(KR_EPI):
                    for ci, (cb, cnb) in enumerate(chunks if KDBG not in ("build", "edge") else []):
                        ncol = cnb * P
                        c0 = cb * P
                        acc = epi.tile([P, cnb, ACC_W], f32, tag="acc")
                        nc.sync.dma_start(
                            out=acc[:],
                            in_=accum[c0:c0 + ncol, :].rearrange("(b p) c -> p b c", p=P))
                        es = epi.tile([P, cnb], f32, tag="es")
                        nc.scalar.activation(es[:], nx2_sb[:, cb:cb + cnb], AF.Exp,
                                             scale=beta128[:, :])
                        denf = epi.tile([P, cnb], f32, tag="denf")
                        nc.vector.tensor_tensor(out=denf[:], in0=acc[:, :, CH:CH + 1].squeeze(2),
                                                in1=padn_sb[:, cb:cb + cnb], op=OP.add)
                        nc.vector.tensor_tensor(out=denf[:], in0=denf[:], in1=es[:], op=OP.add)
                        rec = epi.tile([P, cnb], f32, tag="rec")
                        nc.vector.reciprocal(rec[:], denf[:])
                        hd = h_dm[:, cb:cb + cnb, :]
                        numf = epi.tile([P, cnb, HID], f32, tag="numf")
                        nc.vector.tensor_tensor(
                            out=numf[:], in0=hd,
                            in1=es[:].unsqueeze(2).to_broadcast([P, cnb, HID]), op=OP.mult)
                        nc.vector.tensor_tensor(out=numf[:], in0=numf[:],
                                                in1=acc[:, :, HID:CH], op=OP.add)
                        h1 = epi.tile([P, cnb, HID], f32, tag="h1")
                        nc.vector.tensor_tensor(
                            out=h1[:], in0=numf[:],
                            in1=rec[:].unsqueeze(2).to_broadcast([P, cnb, HID]), op=OP.mult)
                        agg2 = epi.tile([P, cnb, HID], f32, tag="agg2")
                        dv = dinv_sb[:, cb:cb + cnb].unsqueeze(2).to_broadcast([P, cnb, HID])
                        nc.vector.tensor_tensor(out=agg2[:], in0=hd, in1=dv, op=OP.mult)
                        nc.vector.tensor_tensor(out=agg2[:], in0=agg2[:],
                                                in1=acc[:, :, 0:HID], op=OP.add)
                        nc.vector.tensor_tensor(out=agg2[:], in0=agg2[:], in1=dv, op=OP.mult)
                        aggT = epi.tile([HID, ncol], f32, tag="aggT")
                        h1T = epi.tile([HID, ncol], f32, tag="h1T")
                        for b in range(cnb):
                            tp1 = epsum2.tile([HID, P], f32, tag="etp")
                            nc.tensor.transpose(tp1[:], agg2[:, b, :], ident[:, :])
                            nc.scalar.copy(out=aggT[:, b * P:(b + 1) * P], in_=tp1[:])
                            tp2 = epsum2.tile([HID, P], f32, tag="etp")
                            nc.tensor.transpose(tp2[:], h1[:, b, :], ident[:, :])
                            nc.scalar.copy(out=h1T[:, b * P:(b + 1) * P], in_=tp2[:])
                        pf0 = epsum.tile([HID, ncol], f32, tag="mm")
                        nc.tensor.matmul(pf0[:], wsb["wg1"][:], aggT[:], start=True, stop=True)
                        f0T = epi.tile([HID, ncol], f32, tag="f0T")
                        nc.scalar.activation(f0T[:], pf0[:], AF.Identity, bias=bsb["bg1"][:, :])
                        pf1 = epsum.tile([HID, ncol], f32, tag="mm")
                        nc.tensor.matmul(pf1[:], wsb["wg2"][:], aggT[:], start=True, stop=True)
                        f1T = epi.tile([HID, ncol], f32, tag="f1T")
                        nc.scalar.activation(f1T[:], pf1[:], AF.Identity, bias=bsb["bg2"][:, :])
                        pp0 = epsum.tile([HID, ncol], f32, tag="mm")
                        nc.tensor.matmul(pp0[:], wsb["wf"][:], f0T[:], start=True, stop=True)
                        p0T = epi.tile([HID, ncol], f32, tag="p0T")
                        nc.scalar.activation(p0T[:], pp0[:], AF.Tanh, bias=bsb["bf"][:, :])
                        pp1 = epsum.tile([HID, ncol], f32, tag="mm")
                        nc.tensor.matmul(pp1[:], wsb["wf"][:], f1T[:], start=True, stop=True)
                        p1T = epi.tile([HID, ncol], f32, tag="p1T")
                        nc.scalar.activation(p1T[:], pp1[:], AF.Tanh, bias=bsb["bf"][:, :])
                        hTl = epi.tile([HID, ncol], f32, tag="hTl")
                        nc.sync.dma_start(out=hTl[:], in_=hT_d[:, c0:c0 + ncol])
                        ppx = epsum.tile([HID, ncol], f32, tag="mm")
                        nc.tensor.matmul(ppx[:], wsb["wx"][:], hTl[:], start=True, stop=True)
                        xpj = epi.tile([HID, ncol], f32, tag="xpj")
                        nc.scalar.activation(xpj[:], ppx[:], AF.Tanh, bias=bsb["bx"][:, :])
                        t0 = epi.tile([HID, ncol], f32, tag="t0")
                        nc.vector.tensor_tensor(out=t0[:], in0=p0T[:], in1=xpj[:], op=OP.mult)
                        t1 = epi.tile([HID, ncol], f32, tag="t1")
                        nc.vector.scalar_tensor_tensor(
                            out=t1[:], in0=p1T[:], scalar=-1.0, in1=xpj[:],
                            op0=OP.mult, op1=OP.mult)
                        pl = epsum2.tile([1, ncol], f32, tag="psmall")
                        nc.tensor.matmul(pl[:], ones_col[:], t0[:], start=True, stop=False)
                        nc.tensor.matmul(pl[:], ones_col[:], t1[:], start=False, stop=True)
                        s0 = epi.tile([1, ncol], f32, tag="s0")
                        nc.scalar.activation(s0[:], pl[:], AF.Sigmoid)
                        ps0 = epsum.tile([HID, ncol], f32, tag="mm")
                        nc.tensor.matmul(ps0[:], ones_row[:], s0[:], start=True, stop=True)
                        d01 = epi.tile([HID, ncol], f32, tag="d01")
                        nc.vector.tensor_tensor(out=d01[:], in0=f0T[:], in1=f1T[:],
                                                op=OP.subtract)
                        nc.vector.tensor_tensor(out=d01[:], in0=d01[:], in1=ps0[:], op=OP.mult)
                        resT = epi.tile([HID, ncol], f32, tag="resT")
                        nc.vector.tensor_tensor(out=resT[:], in0=d01[:], in1=f1T[:], op=OP.add)
                        py = epsum2.tile([2, ncol], f32, tag="psmall")
                        nc.tensor.matmul(py[:], wc0_sb[:], resT[:], start=True, stop=False)
                        nc.tensor.matmul(py[:], wc1_sb[:], h1T[:], start=False, stop=True)
                        ysb = epi.tile([2, ncol], f32, tag="ysb")
                        nc.scalar.activation(ysb[:], py[:], AF.Identity, bias=bc_sb[:, :])
                        nc.sync.dma_start(out=out[:, c0:c0 + ncol], in_=ysb[:])

    nc.compile()
    return nc


_CACHE = {}


def kernel(**inputs):
    from concourse.bass_utils import run_bass_kernel_spmd

    x = np.asarray(inputs["x"], np.float32)
    edge_index = np.asarray(inputs["edge_index"])
    cores, meta = _prep(x, edge_index)
    structure = meta["structure"]
    gw = cores[0]["gidx"].shape[1]
    xw = cores[0]["xidx"].shape[1]
    sw = cores[0]["sdeg"].shape[1]

    key = (gw, xw, sw, tuple(meta["ZROW"]),
           tuple((s["b0"], s["nb"], s["K"]) for w in structure for s in w))
    if key not in _CACHE:
        _CACHE[key] = _build(structure, meta["ZROW"], gw, xw, sw)
    nc = _CACHE[key]

    shared = {}
    for n in ("w1", "w2", "w3", "wg1", "wg2", "wf", "wx"):
        shared[n] = np.ascontiguousarray(np.asarray(inputs[n], np.float32))
    wc = np.asarray(inputs["wc"], np.float32)
    shared["wc0"] = np.ascontiguousarray(wc[0:HID, :])
    shared["wc1"] = np.ascontiguousarray(wc[HID:2 * HID, :])
    for n in ("b1", "b2", "b3", "bg1", "bg2", "bf", "bx"):
        shared[n] = np.asarray(inputs[n], np.float32).reshape(HID, 1)
    shared["bc"] = np.asarray(inputs["bc"], np.float32).reshape(2, 1)
    shared["beta"] = np.asarray(inputs["beta"], np.float32).reshape(1, 1)

    in_maps = []
    for k in range(NCORE):
        m = dict(shared)
        m["xpT"] = cores[k]["xpT"]
        m["gidx"] = cores[k]["gidx"]
        m["xidx"] = cores[k]["xidx"]
        m["sidx"] = cores[k]["sidx"]
        m["sdegt"] = cores[k]["sdeg"]
        m["dinvt"] = cores[k]["dinvc"]
        m["padnt"] = cores[k]["padneg"]
        in_maps.append(m)

    res = run_bass_kernel_spmd(nc, in_maps, core_ids=list(range(NCORE)))
    _last_run["nc"] = nc
    _last_run["in_maps"] = in_maps

    y = np.zeros((N, 2), np.float32)
    for k in range(NCORE):
        sn = meta["slot_node"][k * DCORE:(k + 1) * DCORE]
        sel = sn >= 0
        y[sn[sel]] = res.results[k]["out"].T[sel]
    return y


# exposed for test harness timing
_last_run = {}



# revision 8
# speedup vs baseline: 2.3193x; 2.3193x over previous
"""Distributed Trainium2 (8-core) kernel for the GCN+AGNN message-passing model.

Strategy (destination-sharded, window-tiled gathers; f16 data path):
- Nodes are degree-sorted and snake-assigned to 8 cores (12544 slots/core incl
  dummies). Each core computes the input MLP for its shard in channel-major
  f16 layout, derives the packed per-node feature row [hn=dinv*h | xn=h/|h|]
  (f16, 256B), with dummy-slot rows forced to exact zeros; an AllGather into a
  Shared-address-space table replicates all 100352 rows once per device group.
- Edges go to the core owning their destination. Because dma_gather indices
  are int16, sources are split into 4 windows of 32768 table rows. Per window,
  local destinations are sorted by in-count, bucketed 128-wide, padded to the
  bucket max K (pads hit an all-zero in-window row). dma_gathers (16 slots =
  2048 indices each, round-robin over all 4 SWDGE queues) fetch
  [128 dests, K slots, 128ch] f16 tiles; VectorE segmented reduces produce GCN
  aggregate / AGNN softmax numerator+denominator partials which
  dma_scatter_add (768B rows, same queue rotation) accumulates into a
  [12545, 192] f32 DRAM accumulator.
- Epilogue (per 512 dests): add analytic self-loop terms, finish the AGNN
  softmax, PE-transpose to channel-major and run the small GCN/projection/
  classifier matmuls in f16 (f32 PSUM); host inverse-permutes the
  [2, 12544] per-core outputs.
"""
import os
import sys

for _p in ("/opt/trn_rl_repo", "/root/.axon_site/_ro/trn_rl_repo"):
    if os.path.isdir(_p) and _p not in sys.path:
        sys.path.insert(0, _p)

import numpy as np

NCORE = 8
N = 100000
HID = 64
CH = 128
P = 128
DCORE = 12544
NBUCK = DCORE // P          # 98
TABLE_ROWS = NCORE * DCORE  # 100352
WIN = 32768
NWIN = 4
SMAX = 64
NBMAX = 8
ACC_W = 192                 # f32 accumulator row -> 768B
ACC_ROWS = DCORE + 1
PAD_DEST = 12500
EPS = 1e-12
CHUNK = 4                   # buckets per epilogue chunk (512 dests)
NCHUNK = NBUCK // CHUNK + (1 if NBUCK % CHUNK else 0)   # 25 (24x4 + 1x2)
GCHUNK = int(os.environ.get("KGCH", "8"))   # slots per dma_gather call
KSHARED = os.environ.get("KSHARED", "1") == "1"


def _chunks():
    out = []
    b = 0
    while b < NBUCK:
        nb = min(CHUNK, NBUCK - b)
        out.append((b, nb))
        b += nb
    return out


def _wrap_idx(idxs):
    n = len(idxs)
    nc16 = (n + 15) // 16
    w = np.zeros((16, nc16), np.int16)
    w[np.arange(n) % 16, np.arange(n) // 16] = idxs
    return np.tile(w, (8, 1))


def _prep(x, edge_index):
    x = np.asarray(x, np.float32)
    row = np.asarray(edge_index[0], np.int64)
    col = np.asarray(edge_index[1], np.int64)
    deg = np.bincount(col, minlength=N).astype(np.int64) + 1
    dinv = (deg.astype(np.float64) ** -0.5).astype(np.float32)
    sdeg = np.sqrt(deg.astype(np.float32))

    c_in = deg - 1
    order = np.argsort(-c_in, kind="stable")
    pos = np.arange(N)
    r, j = pos // NCORE, pos % NCORE
    core_of_pos = np.where(r % 2 == 0, j, NCORE - 1 - j)
    node_slot = np.empty(N, np.int64)
    slot_node = np.full(TABLE_ROWS, -1, np.int64)
    for k in range(NCORE):
        nodes_k = order[core_of_pos == k]
        slots = k * DCORE + np.arange(len(nodes_k))
        node_slot[nodes_k] = slots
        slot_node[slots] = nodes_k

    ZROW = []
    for w in range(NWIN):
        lo, hi = w * WIN, min((w + 1) * WIN, TABLE_ROWS)
        dum = np.where(slot_node[lo:hi] < 0)[0]
        assert len(dum) > 0, f"window {w} has no dummy slot for a zero row"
        ZROW.append(int(lo + dum[0]))

    src_slot = node_slot[row]
    dst_slot = node_slot[col]
    dst_core = dst_slot // DCORE
    dst_local = dst_slot % DCORE
    src_win = src_slot // WIN

    counts = np.zeros((NCORE, NWIN, DCORE), np.int64)
    for k in range(NCORE):
        m = dst_core == k
        for w in range(NWIN):
            counts[k, w] = np.bincount(dst_local[m & (src_win == w)], minlength=DCORE)

    pi = np.zeros((NCORE, NWIN, DCORE), np.int64)
    csort = np.zeros((NCORE, NWIN, DCORE), np.int64)
    for k in range(NCORE):
        for w in range(NWIN):
            o = np.argsort(-counts[k, w], kind="stable")
            pi[k, w] = o
            csort[k, w] = counts[k, w][o]

    structure = []
    for w in range(NWIN):
        nz = int(max((csort[k, w] > 0).sum() for k in range(NCORE)))
        nb_w = (nz + P - 1) // P
        K_b = [int(csort[:, w, b * P].max()) for b in range(nb_w)]
        subs = []
        b = 0
        while b < nb_w:
            K = K_b[b]
            nb = 1
            while (b + nb < nb_w and K_b[b + nb] == K and nb < NBMAX
                   and (nb + 1) * K <= SMAX):
                nb += 1
            subs.append({"b0": b, "nb": nb, "K": K})
            b += nb
        structure.append(subs)

    cores = []
    for k in range(NCORE):
        m = dst_core == k
        es_k = src_slot[m]
        ed_k = dst_local[m]
        gidx_parts, xidx_parts, sidx_parts, sdeg_parts = [], [], [], []
        padcnt = np.zeros(DCORE, np.int64)
        for w in range(NWIN):
            inv_pi = np.empty(DCORE, np.int64)
            inv_pi[pi[k, w]] = np.arange(DCORE)
            mw = (es_k // WIN) == w
            es, ed = es_k[mw], ed_k[mw]
            rank = inv_pi[ed]
            o = np.lexsort((es, rank))
            es, rank = es[o], rank[o]
            bc = np.bincount(rank, minlength=DCORE)
            jj = np.arange(len(rank)) - np.repeat(
                np.concatenate([[0], np.cumsum(bc)[:-1]]), bc)
            for sub in structure[w]:
                b0, nb, K = sub["b0"], sub["nb"], sub["K"]
                S = nb * K
                g = np.full((S, P), ZROW[w] - w * WIN, np.int64)
                sd = np.zeros((P, S), np.float32)
                sel = (rank >= b0 * P) & (rank < (b0 + nb) * P)
                rr, ee, jx = rank[sel], es[sel], jj[sel]
                bi = rr // P - b0
                d = rr % P
                slot = bi * K + jx
                g[slot, d] = ee - w * WIN
                sd[d, slot] = sdeg[slot_node[ee]]
                gidx_parts.append(_wrap_idx(g.reshape(-1).astype(np.int16)))
                sdeg_parts.append(sd.astype(np.float16))
                q = np.arange(b0 * P, (b0 + nb) * P)
                xd = np.where(q < DCORE, pi[k, w][np.minimum(q, DCORE - 1)], PAD_DEST)
                xidx_parts.append(_wrap_idx(xd.astype(np.int16)))
                sidx_parts.append(_wrap_idx(np.where(q < DCORE, xd, ACC_ROWS - 1).astype(np.int16)))
                cw = counts[k, w][xd[q < DCORE]]
                padcnt[xd[q < DCORE]] += K - cw
        ld = (np.arange(NBUCK)[None, :] * P + np.arange(P)[:, None])
        node_of_ld = slot_node[k * DCORE + ld]
        real = node_of_ld >= 0
        dinvc = np.where(real, dinv[np.maximum(node_of_ld, 0)], 0.0).astype(np.float32)
        maskc = real.astype(np.float32)
        padneg = np.where(real, -padcnt[ld].astype(np.float32), 0.0).astype(np.float32)
        xp = np.zeros((DCORE, HID), np.float32)
        sel = slot_node[k * DCORE:(k + 1) * DCORE] >= 0
        xp[sel] = x[slot_node[k * DCORE:(k + 1) * DCORE][sel]]
        cores.append({
            "gidx": np.concatenate(gidx_parts, axis=1),
            "xidx": np.concatenate(xidx_parts, axis=1),
            "sidx": np.concatenate(sidx_parts, axis=1),
            "sdeg": np.concatenate(sdeg_parts, axis=1),
            "dinvc": dinvc, "maskc": maskc, "padneg": padneg,
            "xpT": np.ascontiguousarray(xp.T.astype(np.float16)),
        })
    meta = {"structure": structure, "ZROW": ZROW, "slot_node": slot_node}
    return cores, meta


def _build(structure, zrows, gw, xw, sw):
    """Build the SPMD Bass program. gw/xw/sw: widths of the flat idx/sdeg arrays."""
    KDBG = os.environ.get("KDBG", "")
    KEDGE = int(os.environ.get("KEDGE", "9999"))
    KSKIP = set(os.environ.get("KSKIP", "").split(","))
    KREPS = int(os.environ.get("KREPS", "1"))
    KR_BUILD = int(os.environ.get("KR_BUILD", "1"))
    KR_COLL = int(os.environ.get("KR_COLL", "1"))
    KR_EDGE = int(os.environ.get("KR_EDGE", "1"))
    KR_EPI = int(os.environ.get("KR_EPI", "1"))
    from concourse import bass, bacc, mybir, tile
    from concourse.masks import make_identity

    f32, f16, i16 = mybir.dt.float32, mybir.dt.float16, mybir.dt.int16
    AX = mybir.AxisListType
    OP = mybir.AluOpType
    AF = mybir.ActivationFunctionType

    nc = bacc.Bacc("TRN2", target_bir_lowering=False, debug=False,
                   enable_asserts=False, num_devices=NCORE,
                   num_swdge_queues=4)

    xpT = nc.dram_tensor("xpT", [HID, DCORE], f16, kind="ExternalInput")
    gidx = nc.dram_tensor("gidx", [P, gw], i16, kind="ExternalInput")
    xidx = nc.dram_tensor("xidx", [P, xw], i16, kind="ExternalInput")
    sidx = nc.dram_tensor("sidx", [P, xw], i16, kind="ExternalInput")
    sdegt = nc.dram_tensor("sdegt", [P, sw], f16, kind="ExternalInput")
    dinvt = nc.dram_tensor("dinvt", [P, NBUCK], f32, kind="ExternalInput")
    maskt = nc.dram_tensor("maskt", [P, NBUCK], f32, kind="ExternalInput")
    padnt = nc.dram_tensor("padnt", [P, NBUCK], f32, kind="ExternalInput")
    wnames = ["w1", "w2", "w3", "wg1", "wg2", "wf", "wx"]
    wts = {n: nc.dram_tensor(n, [HID, HID], f16, kind="ExternalInput") for n in wnames}
    wc0t = nc.dram_tensor("wc0", [HID, 2], f16, kind="ExternalInput")
    wc1t = nc.dram_tensor("wc1", [HID, 2], f16, kind="ExternalInput")
    bnames = ["b1", "b2", "b3", "bg1", "bg2", "bf", "bx"]
    bts = {n: nc.dram_tensor(n, [HID, 1], f32, kind="ExternalInput") for n in bnames}
    bct = nc.dram_tensor("bc", [2, 1], f32, kind="ExternalInput")
    betat = nc.dram_tensor("beta", [1, 1], f32, kind="ExternalInput")
    out = nc.dram_tensor("out", [2, DCORE], f32, kind="ExternalOutput")

    shard = nc.dram_tensor("shard", [DCORE, CH], f16)
    table = nc.dram_tensor("table", [TABLE_ROWS, CH], f16,
                           addr_space="Shared" if KSHARED else "Local")
    hT_d = nc.dram_tensor("hT_d", [HID, DCORE], f16)
    accum = nc.dram_tensor("accum", [ACC_ROWS, ACC_W], f32)
    if KDBG == "build":
        dbg = nc.dram_tensor("dbg", [P, CH], f32, kind="ExternalOutput")
    if KDBG == "edge":
        dbga = nc.dram_tensor("dbga", [ACC_ROWS, ACC_W], f32, kind="ExternalOutput")

    chunks = _chunks()

    with tile.TileContext(nc) as tc:
        with tc.tile_pool(name="const", bufs=1) as cpool, \
             tc.tile_pool(name="persist", bufs=1) as ppool:

            # ---- constants ----
            wsb = {}
            for n in wnames:
                t = cpool.tile([HID, HID], f16, name=f"w_{n}")
                nc.sync.dma_start(out=t[:], in_=wts[n][:])
                wsb[n] = t
            wc0_sb = cpool.tile([HID, 2], f16)
            nc.sync.dma_start(out=wc0_sb[:], in_=wc0t[:])
            wc1_sb = cpool.tile([HID, 2], f16)
            nc.sync.dma_start(out=wc1_sb[:], in_=wc1t[:])
            bsb = {}
            for n in bnames:
                t = cpool.tile([HID, 1], f32, name=f"b_{n}")
                nc.sync.dma_start(out=t[:], in_=bts[n][:])
                bsb[n] = t
            bc_sb = cpool.tile([2, 1], f32)
            nc.sync.dma_start(out=bc_sb[:], in_=bct[:])
            beta1 = cpool.tile([1, 1], f32)
            nc.sync.dma_start(out=beta1[:], in_=betat[:])
            beta128 = cpool.tile([P, 1], f32)
            nc.gpsimd.partition_broadcast(beta128[:], beta1[:])
            ident = cpool.tile([P, P], f32)
            make_identity(nc, ident[:])
            ident16 = cpool.tile([P, P], f16)
            nc.scalar.copy(out=ident16[:], in_=ident[:])
            ones_col = cpool.tile([HID, 1], f16)
            nc.vector.memset(ones_col[:], 1.0)
            ones_row = cpool.tile([1, HID], f16)
            nc.vector.memset(ones_row[:], 1.0)
            zacc = cpool.tile([P, ACC_W], f32)
            nc.vector.memset(zacc[:], 0.0)
            epsb = cpool.tile([P, 1], f32)
            nc.vector.memset(epsb[:], float(EPS))
            dinv_sb = ppool.tile([P, NBUCK], f32)
            nc.sync.dma_start(out=dinv_sb[:], in_=dinvt[:])
            mask_sb = ppool.tile([P, NBUCK], f32)
            nc.sync.dma_start(out=mask_sb[:], in_=maskt[:])
            padn_sb = ppool.tile([P, NBUCK], f32)
            nc.sync.dma_start(out=padn_sb[:], in_=padnt[:])
            h_dm = ppool.tile([P, NBUCK, HID], f16)
            nx2_sb = ppool.tile([P, NBUCK], f32)

            for _rep in range(KREPS):
                # ---- zero the accumulator ----
                for r0 in range(0, ACC_ROWS, P):
                    r1 = min(r0 + P, ACC_ROWS)
                    nc.sync.dma_start(out=accum[r0:r1, :], in_=zacc[:r1 - r0, :])

                # ---- build phase ----
                with tc.tile_pool(name="build", bufs=2) as bpool, \
                     tc.tile_pool(name="bpsum", bufs=2, space="PSUM") as bpsum:
                  for _rb in range(KR_BUILD):
                    for ci, (cb, cnb) in enumerate(chunks):
                        ncol = cnb * P
                        c0 = cb * P
                        xc = bpool.tile([HID, ncol], f16, tag="xc")
                        nc.sync.dma_start(out=xc[:], in_=xpT[:, c0:c0 + ncol])
                        pm = bpsum.tile([HID, ncol], f32, tag="bp1")
                        nc.tensor.matmul(pm[:], wsb["w1"][:], xc[:], start=True, stop=True)
                        hh1 = bpool.tile([HID, ncol], f16, tag="hh1")
                        nc.scalar.activation(hh1[:], pm[:], AF.Relu, bias=bsb["b1"][:, :])
                        pm2 = bpsum.tile([HID, ncol], f32, tag="bp2")
                        nc.tensor.matmul(pm2[:], wsb["w2"][:], hh1[:], start=True, stop=True)
                        hh2 = bpool.tile([HID, ncol], f16, tag="hh2")
                        nc.scalar.activation(hh2[:], pm2[:], AF.Relu, bias=bsb["b2"][:, :])
                        pm3 = bpsum.tile([HID, ncol], f32, tag="bp3")
                        nc.tensor.matmul(pm3[:], wsb["w3"][:], hh2[:], start=True, stop=True)
                        hTc = bpool.tile([HID, ncol], f16, tag="hTc")
                        nc.scalar.activation(hTc[:], pm3[:], AF.Identity, bias=bsb["b3"][:, :])
                        nc.sync.dma_start(out=hT_d[:, c0:c0 + ncol], in_=hTc[:])
                        # transpose to dest-major: in [64, 128] -> out [128, 64]
                        for b in range(cnb):
                            tp = bpsum.tile([P, HID], f16, tag="btp")
                            nc.tensor.transpose(tp[:], hTc[:, b * P:(b + 1) * P],
                                                ident16[0:HID, 0:HID])
                            nc.scalar.copy(out=h_dm[:, cb + b, :], in_=tp[:])
                        hd = h_dm[:, cb:cb + cnb, :]
                        sq = bpool.tile([P, cnb, HID], f32, tag="sq")
                        nc.scalar.activation(sq[:], hd, AF.Square)
                        n2 = bpool.tile([P, cnb], f32, tag="n2")
                        nc.vector.tensor_reduce(out=n2[:], in_=sq[:], axis=AX.X, op=OP.add)
                        nrm = bpool.tile([P, cnb], f32, tag="nrm")
                        nc.scalar.activation(nrm[:], n2[:], AF.Sqrt)
                        nrme = bpool.tile([P, cnb], f32, tag="nrme")
                        nc.scalar.activation(nrme[:], nrm[:], AF.Identity, bias=epsb[:, :])
                        rn = bpool.tile([P, cnb], f32, tag="rn")
                        nc.vector.reciprocal(rn[:], nrme[:])
                        nx = bpool.tile([P, cnb], f32, tag="nx")
                        nc.vector.tensor_tensor(out=nx[:], in0=nrm[:], in1=rn[:], op=OP.mult)
                        nc.scalar.activation(nx2_sb[:, cb:cb + cnb], nx[:], AF.Square)
                        rnm = bpool.tile([P, cnb], f32, tag="rnm")
                        nc.vector.tensor_tensor(out=rnm[:], in0=rn[:],
                                                in1=mask_sb[:, cb:cb + cnb], op=OP.mult)
                        st = bpool.tile([P, cnb, CH], f16, tag="st")
                        nc.vector.tensor_tensor(
                            out=st[:, :, 0:HID], in0=hd,
                            in1=dinv_sb[:, cb:cb + cnb].unsqueeze(2).to_broadcast([P, cnb, HID]),
                            op=OP.mult)
                        nc.vector.tensor_tensor(
                            out=st[:, :, HID:CH], in0=hd,
                            in1=rnm[:].unsqueeze(2).to_broadcast([P, cnb, HID]),
                            op=OP.mult)
                        nc.sync.dma_start(
                            out=shard[c0:c0 + ncol, :].rearrange("(b p) c -> p b c", p=P),
                            in_=st[:])

                # ---- all-gather the table (dummy rows are exact zeros) ----
                for _rc in range(KR_COLL):
                    nc.gpsimd.collective_compute(
                        "AllGather", mybir.AluOpType.bypass,
                        replica_groups=[list(range(NCORE))],
                        ins=[shard[:, :]], outs=[table[:, :]])

                if KDBG == "build":
                    with tc.tile_pool(name="dbgp", bufs=1) as dp:
                        dt_ = dp.tile([P, CH], f16)
                        nc.sync.dma_start(out=dt_[:], in_=table[50000:50000 + P, :])
                        dt32 = dp.tile([P, CH], f32)
                        nc.scalar.copy(out=dt32[:], in_=dt_[:])
                        nc.sync.dma_start(out=dbg[:], in_=dt32[:])
                # ---- edge phase ----
                with tc.tile_pool(name="io", bufs=4) as io, \
                     tc.tile_pool(name="edge", bufs=3) as ep:
                  for _re in range(KR_EDGE):
                    g16 = x16 = soff = 0
                    nsub = 0
                    qrr = [0]
                    for w in (range(NWIN) if KDBG != "build" else []):
                        base = w * WIN
                        bend = min((w + 1) * WIN, TABLE_ROWS)
                        for sub in structure[w]:
                            nb, K = sub["nb"], sub["K"]
                            S = nb * K
                            nsub += 1
                            if nsub > KEDGE:
                                g16 += S * 8
                                x16 += nb * 8
                                soff += S
                                continue
                            it = io.tile([P, S * 8], i16, tag="it")
                            nc.sync.dma_start(out=it[:], in_=gidx[:, g16:g16 + S * 8])
                            xt = io.tile([P, nb * 8], i16, tag="xt")
                            nc.sync.dma_start(out=xt[:], in_=xidx[:, x16:x16 + nb * 8])
                            stx = io.tile([P, nb * 8], i16, tag="stx")
                            nc.sync.dma_start(out=stx[:], in_=sidx[:, x16:x16 + nb * 8])
                            sd = io.tile([P, S], f16, tag="sd")
                            nc.sync.dma_start(out=sd[:], in_=sdegt[:, soff:soff + S])
                            g16 += S * 8
                            x16 += nb * 8
                            soff += S

                            gt = ep.tile([P, S, CH], f16, tag="gt")
                            if "gt" in KSKIP:
                                nc.vector.memset(gt[:, :, :], 0.0)
                            else:
                                for j0 in range(0, S, GCHUNK):
                                    ns = min(GCHUNK, S - j0)
                                    nc.gpsimd.dma_gather(
                                        out_ap=gt[:, j0:j0 + ns, :],
                                        in_ap=table[base:bend, :],
                                        idxs_ap=it[:, j0 * 8:(j0 + ns) * 8],
                                        num_idxs=ns * P, num_idxs_reg=ns * P,
                                        elem_size=CH, queue_num=qrr[0] % 4)
                                    qrr[0] += 1
                            xc2 = ep.tile([P, nb, CH], f16, tag="xc2")
                            if "xc" in KSKIP:
                                nc.vector.memset(xc2[:, :, :], 0.0)
                            else:
                                nc.gpsimd.dma_gather(
                                    out_ap=xc2[:, :, :], in_ap=shard[:, :],
                                    idxs_ap=xt[:, :], num_idxs=nb * P, num_idxs_reg=nb * P,
                                    elem_size=CH, queue_num=qrr[0] % 4)
                                qrr[0] += 1

                            gat = gt[:, :, :].rearrange("p (b k) c -> p b k c", b=nb)
                            tmpa = ep.tile([P, S, HID], f16, tag="tmpa")
                            nc.vector.tensor_tensor(
                                out=tmpa[:, :, :].rearrange("p (b k) c -> p b k c", b=nb),
                                in0=gat[:, :, :, HID:CH],
                                in1=xc2[:, :, HID:CH].unsqueeze(2).to_broadcast([P, nb, K, HID]),
                                op=OP.mult)
                            alpha = ep.tile([P, S], f16, tag="alpha")
                            with nc.allow_low_precision("cosine logits bounded by 1"):
                                nc.vector.tensor_reduce(out=alpha[:], in_=tmpa[:, :, :],
                                                        axis=AX.X, op=OP.add)
                            e = ep.tile([P, S], f16, tag="e")
                            nc.scalar.activation(e[:], alpha[:], AF.Exp, scale=beta128[:, :])
                            epw = ep.tile([P, S], f16, tag="epw")
                            nc.vector.tensor_tensor(out=epw[:], in0=e[:], in1=sd[:], op=OP.mult)
                            tmpn = ep.tile([P, S, HID], f16, tag="tmpn")
                            nc.vector.tensor_tensor(
                                out=tmpn[:, :, :], in0=gt[:, :, 0:HID],
                                in1=epw[:].unsqueeze(2).to_broadcast([P, S, HID]),
                                op=OP.mult)
                            part = ep.tile([P, nb, ACC_W], f32, tag="part")
                            nc.vector.memset(part[:, :, CH + 1:ACC_W], 0.0)
                            nc.vector.tensor_reduce(
                                out=part[:, :, CH:CH + 1],
                                in_=e[:].rearrange("p (b k) -> p b k", b=nb),
                                axis=AX.X, op=OP.add)
                            # segment-sum gat hn / tmpn over k by in-place
                            # contiguous halving adds (strided tensor_reduce
                            # runs at 1 elem/cycle; this stays in 16-bit 2x)
                            tmn = tmpn[:, :, :].rearrange("p (b k) c -> p b k c", b=nb)
                            for view, dst in ((gat, part[:, :, 0:HID]),
                                              (tmn, part[:, :, HID:CH])):
                                k = K
                                while k > 2:
                                    h = (k + 1) // 2
                                    nc.vector.tensor_tensor(
                                        out=view[:, :, 0:k - h, 0:HID],
                                        in0=view[:, :, 0:k - h, 0:HID],
                                        in1=view[:, :, h:k, 0:HID], op=OP.add)
                                    k = h
                                if k == 2:
                                    nc.vector.tensor_tensor(
                                        out=dst, in0=view[:, :, 0, 0:HID],
                                        in1=view[:, :, 1, 0:HID], op=OP.add)
                                else:
                                    nc.vector.tensor_scalar_add(
                                        dst, view[:, :, 0, 0:HID], 0.0)
                            if "sc" not in KSKIP:
                                nc.gpsimd.dma_scatter_add(
                                    out_ap=accum[:, :], in_ap=part[:, :, :], idxs_ap=stx[:, :],
                                    num_idxs=nb * P, num_idxs_reg=nb * P, elem_size=ACC_W,
                                    queue_num=qrr[0] % 4)
                                qrr[0] += 1

                if KDBG == "edge":
                    with tc.tile_pool(name="dbgp2", bufs=1) as dp2:
                        for r0 in range(0, ACC_ROWS, P):
                            r1 = min(r0 + P, ACC_ROWS)
                            da = dp2.tile([P, ACC_W], f32, tag="da")
                            nc.sync.dma_start(out=da[:r1 - r0, :], in_=accum[r0:r1, :])
                            nc.sync.dma_start(out=dbga[r0:r1, :], in_=da[:r1 - r0, :])
                # ---- epilogue ----
                with tc.tile_pool(name="epi", bufs=2) as epi, \
                     tc.tile_pool(name="epsum", bufs=3, space="PSUM") as epsum, \
                     tc.tile_pool(name="epsum2", bufs=2, space="PSUM") as epsum2:
                  for _rp in range(KR_EPI):
                    for ci, (cb, cnb) in enumerate(chunks if KDBG not in ("build", "edge") else []):
                        ncol = cnb * P
                        c0 = cb * P
                        acc = epi.tile([P, cnb, ACC_W], f32, tag="acc")
                        nc.sync.dma_start(
                            out=acc[:],
                            in_=accum[c0:c0 + ncol, :].rearrange("(b p) c -> p b c", p=P))
                        es = epi.tile([P, cnb], f32, tag="es")
                        nc.scalar.activation(es[:], nx2_sb[:, cb:cb + cnb], AF.Exp,
                                             scale=beta128[:, :])
                        denf = epi.tile([P, cnb], f32, tag="denf")
                        nc.vector.tensor_tensor(out=denf[:], in0=acc[:, :, CH:CH + 1].squeeze(2),
                                                in1=padn_sb[:, cb:cb + cnb], op=OP.add)
                        nc.vector.tensor_tensor(out=denf[:], in0=denf[:], in1=es[:], op=OP.add)
                        rec = epi.tile([P, cnb], f32, tag="rec")
                        nc.vector.reciprocal(rec[:], denf[:])
                        hd = h_dm[:, cb:cb + cnb, :]
                        numf = epi.tile([P, cnb, HID], f32, tag="numf")
                        nc.vector.tensor_tensor(
                            out=numf[:], in0=hd,
                            in1=es[:].unsqueeze(2).to_broadcast([P, cnb, HID]), op=OP.mult)
                        nc.vector.tensor_tensor(out=numf[:], in0=numf[:],
                                                in1=acc[:, :, HID:CH], op=OP.add)
                        h1 = epi.tile([P, cnb, HID], f16, tag="h1")
                        nc.vector.tensor_tensor(
                            out=h1[:], in0=numf[:],
                            in1=rec[:].unsqueeze(2).to_broadcast([P, cnb, HID]), op=OP.mult)
                        ag2f = epi.tile([P, cnb, HID], f32, tag="ag2f")
                        dv = dinv_sb[:, cb:cb + cnb].unsqueeze(2).to_broadcast([P, cnb, HID])
                        nc.vector.tensor_tensor(out=ag2f[:], in0=hd, in1=dv, op=OP.mult)
                        nc.vector.tensor_tensor(out=ag2f[:], in0=ag2f[:],
                                                in1=acc[:, :, 0:HID], op=OP.add)
                        agg2 = epi.tile([P, cnb, HID], f16, tag="agg2")
                        nc.vector.tensor_tensor(out=agg2[:], in0=ag2f[:], in1=dv, op=OP.mult)
                        aggT = epi.tile([HID, ncol], f16, tag="aggT")
                        h1T = epi.tile([HID, ncol], f16, tag="h1T")
                        for b in range(cnb):
                            tp1 = epsum2.tile([HID, P], f16, tag="etp")
                            nc.tensor.transpose(tp1[:], agg2[:, b, :], ident16[:, :])
                            nc.scalar.copy(out=aggT[:, b * P:(b + 1) * P], in_=tp1[:])
                            tp2 = epsum2.tile([HID, P], f16, tag="etp")
                            nc.tensor.transpose(tp2[:], h1[:, b, :], ident16[:, :])
                            nc.scalar.copy(out=h1T[:, b * P:(b + 1) * P], in_=tp2[:])
                        pf0 = epsum.tile([HID, ncol], f32, tag="mm")
                        nc.tensor.matmul(pf0[:], wsb["wg1"][:], aggT[:], start=True, stop=True)
                        f0T = epi.tile([HID, ncol], f16, tag="f0T")
                        nc.scalar.activation(f0T[:], pf0[:], AF.Identity, bias=bsb["bg1"][:, :])
                        pf1 = epsum.tile([HID, ncol], f32, tag="mm")
                        nc.tensor.matmul(pf1[:], wsb["wg2"][:], aggT[:], start=True, stop=True)
                        f1T = epi.tile([HID, ncol], f16, tag="f1T")
                        nc.scalar.activation(f1T[:], pf1[:], AF.Identity, bias=bsb["bg2"][:, :])
                        pp0 = epsum.tile([HID, ncol], f32, tag="mm")
                        nc.tensor.matmul(pp0[:], wsb["wf"][:], f0T[:], start=True, stop=True)
                        p0T = epi.tile([HID, ncol], f16, tag="p0T")
                        nc.scalar.activation(p0T[:], pp0[:], AF.Tanh, bias=bsb["bf"][:, :])
                        pp1 = epsum.tile([HID, ncol], f32, tag="mm")
                        nc.tensor.matmul(pp1[:], wsb["wf"][:], f1T[:], start=True, stop=True)
                        p1T = epi.tile([HID, ncol], f16, tag="p1T")
                        nc.scalar.activation(p1T[:], pp1[:], AF.Tanh, bias=bsb["bf"][:, :])
                        hTl = epi.tile([HID, ncol], f16, tag="hTl")
                        nc.sync.dma_start(out=hTl[:], in_=hT_d[:, c0:c0 + ncol])
                        ppx = epsum.tile([HID, ncol], f32, tag="mm")
                        nc.tensor.matmul(ppx[:], wsb["wx"][:], hTl[:], start=True, stop=True)
                        xpj = epi.tile([HID, ncol], f16, tag="xpj")
                        nc.scalar.activation(xpj[:], ppx[:], AF.Tanh, bias=bsb["bx"][:, :])
                        t0 = epi.tile([HID, ncol], f16, tag="t0")
                        nc.vector.tensor_tensor(out=t0[:], in0=p0T[:], in1=xpj[:], op=OP.mult)
                        t1 = epi.tile([HID, ncol], f16, tag="t1")
                        nc.vector.scalar_tensor_tensor(
                            out=t1[:], in0=p1T[:], scalar=-1.0, in1=xpj[:],
                            op0=OP.mult, op1=OP.mult)
                        pl = epsum2.tile([1, ncol], f32, tag="psmall")
                        nc.tensor.matmul(pl[:], ones_col[:], t0[:], start=True, stop=False)
                        nc.tensor.matmul(pl[:], ones_col[:], t1[:], start=False, stop=True)
                        s0 = epi.tile([1, ncol], f16, tag="s0")
                        nc.scalar.activation(s0[:], pl[:], AF.Sigmoid)
                        ps0 = epsum.tile([HID, ncol], f32, tag="mm")
                        nc.tensor.matmul(ps0[:], ones_row[:], s0[:], start=True, stop=True)
                        d01 = epi.tile([HID, ncol], f16, tag="d01")
                        nc.vector.tensor_tensor(out=d01[:], in0=f0T[:], in1=f1T[:],
                                                op=OP.subtract)
                        nc.vector.tensor_tensor(out=d01[:], in0=d01[:], in1=ps0[:], op=OP.mult)
                        resT = epi.tile([HID, ncol], f16, tag="resT")
                        nc.vector.tensor_tensor(out=resT[:], in0=d01[:], in1=f1T[:], op=OP.add)
                        py = epsum2.tile([2, ncol], f32, tag="psmall")
                        nc.tensor.matmul(py[:], wc0_sb[:], resT[:], start=True, stop=False)
                        nc.tensor.matmul(py[:], wc1_sb[:], h1T[:], start=False, stop=True)
                        ysb = epi.tile([2, ncol], f32, tag="ysb")
                        nc.scalar.activation(ysb[:], py[:], AF.Identity, bias=bc_sb[:, :])
                        nc.sync.dma_start(out=out[:, c0:c0 + ncol], in_=ysb[:])

    nc.compile()
    return nc


_CACHE = {}


def kernel(**inputs):
    from concourse.bass_utils import run_bass_kernel_spmd

    x = np.asarray(inputs["x"], np.float32)
    edge_index = np.asarray(inputs["edge_index"])
    cores, meta = _prep(x, edge_index)
    structure = meta["structure"]
    gw = cores[0]["gidx"].shape[1]
    xw = cores[0]["xidx"].shape[1]
    sw = cores[0]["sdeg"].shape[1]

    key = (gw, xw, sw, tuple(meta["ZROW"]),
           tuple((s["b0"], s["nb"], s["K"]) for w in structure for s in w))
    if key not in _CACHE:
        _CACHE[key] = _build(structure, meta["ZROW"], gw, xw, sw)
    nc = _CACHE[key]

    shared = {}
    for n in ("w1", "w2", "w3", "wg1", "wg2", "wf", "wx"):
        shared[n] = np.ascontiguousarray(np.asarray(inputs[n], np.float16))
    wc = np.asarray(inputs["wc"], np.float32)
    shared["wc0"] = np.ascontiguousarray(wc[0:HID, :]).astype(np.float16)
    shared["wc1"] = np.ascontiguousarray(wc[HID:2 * HID, :]).astype(np.float16)
    for n in ("b1", "b2", "b3", "bg1", "bg2", "bf", "bx"):
        shared[n] = np.asarray(inputs[n], np.float32).reshape(HID, 1)
    shared["bc"] = np.asarray(inputs["bc"], np.float32).reshape(2, 1)
    shared["beta"] = np.asarray(inputs["beta"], np.float32).reshape(1, 1)

    in_maps = []
    for k in range(NCORE):
        m = dict(shared)
        m["xpT"] = cores[k]["xpT"]
        m["gidx"] = cores[k]["gidx"]
        m["xidx"] = cores[k]["xidx"]
        m["sidx"] = cores[k]["sidx"]
        m["sdegt"] = cores[k]["sdeg"]
        m["dinvt"] = cores[k]["dinvc"]
        m["maskt"] = cores[k]["maskc"]
        m["padnt"] = cores[k]["padneg"]
        in_maps.append(m)

    res = run_bass_kernel_spmd(nc, in_maps, core_ids=list(range(NCORE)))
    _last_run["nc"] = nc
    _last_run["in_maps"] = in_maps

    y = np.zeros((N, 2), np.float32)
    for k in range(NCORE):
        sn = meta["slot_node"][k * DCORE:(k + 1) * DCORE]
        sel = sn >= 0
        y[sn[sel]] = res.results[k]["out"].T[sel]
    return y


# exposed for test harness timing
_last_run = {}


# revision 17
# speedup vs baseline: 3.0961x; 1.3349x over previous
"""Distributed Trainium2 (8-core) kernel for the GCN+AGNN message-passing model.

Strategy (destination-sharded, window-tiled gathers; f16 data path):
- Nodes are degree-sorted and snake-assigned to 8 cores (12544 slots/core incl
  dummies). Each core computes the input MLP for its shard in channel-major
  f16 layout, derives the packed per-node feature row [hn=dinv*h | xn=h/|h|]
  (f16, 256B), with dummy-slot rows forced to exact zeros; an AllGather into a
  Shared-address-space table replicates all 100352 rows once per device group.
- Edges go to the core owning their destination. Because dma_gather indices
  are int16, sources are split into 4 windows of 32768 table rows. Per window,
  local destinations are sorted by in-count, bucketed 128-wide, padded to the
  bucket max K (pads hit an all-zero in-window row). dma_gathers (16 slots =
  2048 indices each, round-robin over all 4 SWDGE queues) fetch
  [128 dests, K slots, 128ch] f16 tiles; VectorE segmented reduces produce GCN
  aggregate / AGNN softmax numerator+denominator partials which
  dma_scatter_add (768B rows, same queue rotation) accumulates into a
  [12545, 192] f32 DRAM accumulator.
- Epilogue (per 512 dests): add analytic self-loop terms, finish the AGNN
  softmax, PE-transpose to channel-major and run the small GCN/projection/
  classifier matmuls in f16 (f32 PSUM); host inverse-permutes the
  [2, 12544] per-core outputs.
"""
import os
import sys

for _p in ("/opt/trn_rl_repo", "/root/.axon_site/_ro/trn_rl_repo"):
    if os.path.isdir(_p) and _p not in sys.path:
        sys.path.insert(0, _p)

import numpy as np

NCORE = 8
N = 100000
HID = 64
CH = 128
P = 128
DCORE = 12544
NBUCK = DCORE // P          # 98
TABLE_ROWS = NCORE * DCORE  # 100352
WIN = 32768
NWIN = 4
SMAX = 64
NBMAX = 8
ACC_W = 192                 # f32 accumulator row -> 768B
ACC_ROWS = DCORE + 1
PAD_DEST = 12500
EPS = 1e-12
CHUNK = 4                   # buckets per epilogue chunk (512 dests)
NCHUNK = NBUCK // CHUNK + (1 if NBUCK % CHUNK else 0)   # 25 (24x4 + 1x2)
GCHUNK = int(os.environ.get("KGCH", "8"))   # slots per dma_gather call
KSHARED = os.environ.get("KSHARED", "1") == "1"


def _rho(s):
    """Slot -> shard/accum/table row: partition-major (row = p*NBUCK + b) so
    chunked DMAs are contiguous per partition (fat descriptors)."""
    return (s % P) * NBUCK + s // P


def _chunks():
    out = []
    b = 0
    while b < NBUCK:
        nb = min(CHUNK, NBUCK - b)
        out.append((b, nb))
        b += nb
    return out


def _wrap_idx(idxs):
    n = len(idxs)
    nc16 = (n + 15) // 16
    w = np.zeros((16, nc16), np.int16)
    w[np.arange(n) % 16, np.arange(n) // 16] = idxs
    return np.tile(w, (8, 1))


def _prep(x, edge_index):
    x = np.asarray(x, np.float32)
    row = np.asarray(edge_index[0], np.int64)
    col = np.asarray(edge_index[1], np.int64)
    deg = np.bincount(col, minlength=N).astype(np.int64) + 1
    dinv = (deg.astype(np.float64) ** -0.5).astype(np.float32)
    sdeg = np.sqrt(deg.astype(np.float32))

    c_in = deg - 1
    order = np.argsort(-c_in, kind="stable")
    pos = np.arange(N)
    r, j = pos // NCORE, pos % NCORE
    core_of_pos = np.where(r % 2 == 0, j, NCORE - 1 - j)
    node_slot = np.empty(N, np.int64)
    slot_node = np.full(TABLE_ROWS, -1, np.int64)
    for k in range(NCORE):
        nodes_k = order[core_of_pos == k]
        slots = k * DCORE + np.arange(len(nodes_k))
        node_slot[nodes_k] = slots
        slot_node[slots] = nodes_k

    # table/shard/accum rows use the partition-major permutation rho
    rho_all = _rho(np.arange(DCORE))
    node_row = (node_slot // DCORE) * DCORE + rho_all[node_slot % DCORE]
    row_node = np.full(TABLE_ROWS, -1, np.int64)
    row_node[(np.arange(TABLE_ROWS) // DCORE) * DCORE
             + rho_all[np.arange(TABLE_ROWS) % DCORE]] = slot_node

    ZROW = []
    for w in range(NWIN):
        lo, hi = w * WIN, min((w + 1) * WIN, TABLE_ROWS)
        dum = np.where(row_node[lo:hi] < 0)[0]
        assert len(dum) > 0, f"window {w} has no dummy row for padding"
        ZROW.append(int(lo + dum[0]))

    src_row = node_row[row]
    dst_slot = node_slot[col]
    dst_core = dst_slot // DCORE
    dst_local = dst_slot % DCORE
    src_win = src_row // WIN

    counts = np.zeros((NCORE, NWIN, DCORE), np.int64)
    for k in range(NCORE):
        m = dst_core == k
        for w in range(NWIN):
            counts[k, w] = np.bincount(dst_local[m & (src_win == w)], minlength=DCORE)

    pi = np.zeros((NCORE, NWIN, DCORE), np.int64)
    csort = np.zeros((NCORE, NWIN, DCORE), np.int64)
    for k in range(NCORE):
        for w in range(NWIN):
            o = np.argsort(-counts[k, w], kind="stable")
            pi[k, w] = o
            csort[k, w] = counts[k, w][o]

    structure = []
    for w in range(NWIN):
        nz = int(max((csort[k, w] > 0).sum() for k in range(NCORE)))
        nb_w = (nz + P - 1) // P
        K_b = [int(csort[:, w, b * P].max()) for b in range(nb_w)]
        subs = []
        b = 0
        while b < nb_w:
            K = K_b[b]
            nb = 1
            while (b + nb < nb_w and K_b[b + nb] == K and nb < NBMAX
                   and (nb + 1) * K <= SMAX):
                nb += 1
            subs.append({"b0": b, "nb": nb, "K": K})
            b += nb
        structure.append(subs)

    cores = []
    for k in range(NCORE):
        m = dst_core == k
        es_k = src_row[m]
        ed_k = dst_local[m]
        gidx_parts, xidx_parts, sidx_parts, sdeg_parts = [], [], [], []
        padcnt = np.zeros(DCORE, np.int64)
        for w in range(NWIN):
            inv_pi = np.empty(DCORE, np.int64)
            inv_pi[pi[k, w]] = np.arange(DCORE)
            mw = (es_k // WIN) == w
            es, ed = es_k[mw], ed_k[mw]
            rank = inv_pi[ed]
            o = np.lexsort((es, rank))
            es, rank = es[o], rank[o]
            bc = np.bincount(rank, minlength=DCORE)
            jj = np.arange(len(rank)) - np.repeat(
                np.concatenate([[0], np.cumsum(bc)[:-1]]), bc)
            for sub in structure[w]:
                b0, nb, K = sub["b0"], sub["nb"], sub["K"]
                S = nb * K
                g = np.full((S, P), ZROW[w] - w * WIN, np.int64)
                sd = np.zeros((P, S), np.float32)
                sel = (rank >= b0 * P) & (rank < (b0 + nb) * P)
                rr, ee, jx = rank[sel], es[sel], jj[sel]
                bi = rr // P - b0
                d = rr % P
                slot = bi * K + jx
                g[slot, d] = ee - w * WIN
                sd[d, slot] = sdeg[row_node[ee]]
                gidx_parts.append(_wrap_idx(g.reshape(-1).astype(np.int16)))
                sdeg_parts.append(sd.astype(np.float16))
                q = np.arange(b0 * P, (b0 + nb) * P)
                xd = np.where(q < DCORE, pi[k, w][np.minimum(q, DCORE - 1)], PAD_DEST)
                xidx_parts.append(_wrap_idx(rho_all[xd].astype(np.int16)))
                sidx_parts.append(_wrap_idx(np.where(q < DCORE, rho_all[xd],
                                                     ACC_ROWS - 1).astype(np.int16)))
                cw = counts[k, w][xd[q < DCORE]]
                padcnt[xd[q < DCORE]] += K - cw
        ld = (np.arange(NBUCK)[None, :] * P + np.arange(P)[:, None])
        node_of_ld = slot_node[k * DCORE + ld]
        real = node_of_ld >= 0
        dinvc = np.where(real, dinv[np.maximum(node_of_ld, 0)], 0.0).astype(np.float32)
        maskc = real.astype(np.float32)
        padneg = np.where(real, -padcnt[ld].astype(np.float32), 0.0).astype(np.float32)
        xp = np.zeros((DCORE, HID), np.float32)
        sel = slot_node[k * DCORE:(k + 1) * DCORE] >= 0
        xp[sel] = x[slot_node[k * DCORE:(k + 1) * DCORE][sel]]
        cores.append({
            "gidx": np.concatenate(gidx_parts, axis=1),
            "xidx": np.concatenate(xidx_parts, axis=1),
            "sidx": np.concatenate(sidx_parts, axis=1),
            "sdeg": np.concatenate(sdeg_parts, axis=1),
            "dinvc": dinvc, "maskc": maskc, "padneg": padneg,
            "xpT": np.ascontiguousarray(xp.T.astype(np.float16)),
        })
    meta = {"structure": structure, "ZROW": ZROW, "slot_node": slot_node}
    return cores, meta


def _build(structure, zrows, gw, xw, sw):
    """Build the SPMD Bass program. gw/xw/sw: widths of the flat idx/sdeg arrays."""
    KDBG = os.environ.get("KDBG", "")
    KEDGE = int(os.environ.get("KEDGE", "9999"))
    KSKIP = set(os.environ.get("KSKIP", "").split(","))
    KREPS = int(os.environ.get("KREPS", "1"))
    KR_BUILD = int(os.environ.get("KR_BUILD", "1"))
    KR_COLL = int(os.environ.get("KR_COLL", "1"))
    KR_EDGE = int(os.environ.get("KR_EDGE", "1"))
    KR_EPI = int(os.environ.get("KR_EPI", "1"))
    from concourse import bass, bacc, mybir, tile
    from concourse.masks import make_identity

    f32, f16, i16 = mybir.dt.float32, mybir.dt.float16, mybir.dt.int16
    AX = mybir.AxisListType
    OP = mybir.AluOpType
    AF = mybir.ActivationFunctionType

    nc = bacc.Bacc("TRN2", target_bir_lowering=False, debug=False,
                   enable_asserts=False, num_devices=NCORE,
                   num_swdge_queues=4)

    xpT = nc.dram_tensor("xpT", [HID, DCORE], f16, kind="ExternalInput")
    gidx = nc.dram_tensor("gidx", [P, gw], i16, kind="ExternalInput")
    xidx = nc.dram_tensor("xidx", [P, xw], i16, kind="ExternalInput")
    sidx = nc.dram_tensor("sidx", [P, xw], i16, kind="ExternalInput")
    sdegt = nc.dram_tensor("sdegt", [P, sw], f16, kind="ExternalInput")
    dinvt = nc.dram_tensor("dinvt", [P, NBUCK], f32, kind="ExternalInput")
    maskt = nc.dram_tensor("maskt", [P, NBUCK], f32, kind="ExternalInput")
    padnt = nc.dram_tensor("padnt", [P, NBUCK], f32, kind="ExternalInput")
    wnames = ["w1", "w2", "w3", "wg1", "wg2", "wf", "wx"]
    wts = {n: nc.dram_tensor(n, [HID, HID], f16, kind="ExternalInput") for n in wnames}
    wc0t = nc.dram_tensor("wc0", [HID, 2], f16, kind="ExternalInput")
    wc1t = nc.dram_tensor("wc1", [HID, 2], f16, kind="ExternalInput")
    bnames = ["b1", "b2", "b3", "bg1", "bg2", "bf", "bx"]
    bts = {n: nc.dram_tensor(n, [HID, 1], f32, kind="ExternalInput") for n in bnames}
    bct = nc.dram_tensor("bc", [2, 1], f32, kind="ExternalInput")
    betat = nc.dram_tensor("beta", [1, 1], f32, kind="ExternalInput")
    out = nc.dram_tensor("out", [2, DCORE], f32, kind="ExternalOutput")

    shard = nc.dram_tensor("shard", [DCORE, CH], f16)
    table = nc.dram_tensor("table", [TABLE_ROWS, CH], f16,
                           addr_space="Shared" if KSHARED else "Local")
    hT_d = nc.dram_tensor("hT_d", [HID, DCORE], f16)
    accum = nc.dram_tensor("accum", [ACC_ROWS, ACC_W], f32)
    if KDBG == "build":
        dbg = nc.dram_tensor("dbg", [P, CH], f32, kind="ExternalOutput")
    if KDBG == "edge":
        dbga = nc.dram_tensor("dbga", [ACC_ROWS, ACC_W], f32, kind="ExternalOutput")

    chunks = _chunks()

    with tile.TileContext(nc) as tc:
        with tc.tile_pool(name="const", bufs=1) as cpool, \
             tc.tile_pool(name="persist", bufs=1) as ppool:

            # ---- constants ----
            wsb = {}
            for n in wnames:
                t = cpool.tile([HID, HID], f16, name=f"w_{n}")
                nc.sync.dma_start(out=t[:], in_=wts[n][:])
                wsb[n] = t
            wc0_sb = cpool.tile([HID, 2], f16)
            nc.sync.dma_start(out=wc0_sb[:], in_=wc0t[:])
            wc1_sb = cpool.tile([HID, 2], f16)
            nc.sync.dma_start(out=wc1_sb[:], in_=wc1t[:])
            bsb = {}
            for n in bnames:
                t = cpool.tile([HID, 1], f32, name=f"b_{n}")
                nc.sync.dma_start(out=t[:], in_=bts[n][:])
                bsb[n] = t
            bc_sb = cpool.tile([2, 1], f32)
            nc.sync.dma_start(out=bc_sb[:], in_=bct[:])
            beta1 = cpool.tile([1, 1], f32)
            nc.sync.dma_start(out=beta1[:], in_=betat[:])
            beta128 = cpool.tile([P, 1], f32)
            nc.gpsimd.partition_broadcast(beta128[:], beta1[:])
            ident = cpool.tile([P, P], f32)
            make_identity(nc, ident[:])
            ident16 = cpool.tile([P, P], f16)
            nc.scalar.copy(out=ident16[:], in_=ident[:])
            ones_col = cpool.tile([HID, 1], f16)
            nc.vector.memset(ones_col[:], 1.0)
            ones_row = cpool.tile([1, HID], f16)
            nc.vector.memset(ones_row[:], 1.0)
            zacc = cpool.tile([P, ACC_W], f32)
            nc.vector.memset(zacc[:], 0.0)
            zacc2 = cpool.tile([P, 14 * ACC_W], f32)
            nc.vector.memset(zacc2[:], 0.0)
            epsb = cpool.tile([P, 1], f32)
            nc.vector.memset(epsb[:], float(EPS))
            dinv_sb = ppool.tile([P, NBUCK], f32)
            nc.sync.dma_start(out=dinv_sb[:], in_=dinvt[:])
            mask_sb = ppool.tile([P, NBUCK], f32)
            nc.sync.dma_start(out=mask_sb[:], in_=maskt[:])
            padn_sb = ppool.tile([P, NBUCK], f32)
            nc.sync.dma_start(out=padn_sb[:], in_=padnt[:])
            h_dm = ppool.tile([P, NBUCK, HID], f16)
            nx2_sb = ppool.tile([P, NBUCK], f32)

            for _rep in range(KREPS):
                # ---- zero the accumulator (7 fat DMAs of 14 rows/partition) ----
                accz = accum[0:DCORE, :].rearrange("(p a i) c -> p a (i c)", p=P, a=7)
                for a in range(7):
                    nc.sync.dma_start(out=accz[:, a, :], in_=zacc2[:])
                nc.sync.dma_start(out=accum[DCORE:ACC_ROWS, :], in_=zacc[0:1, :])

                # ---- build phase ----
                with tc.tile_pool(name="build", bufs=2) as bpool, \
                     tc.tile_pool(name="bpsum", bufs=2, space="PSUM") as bpsum:
                  for _rb in range(KR_BUILD):
                    for ci, (cb, cnb) in enumerate(chunks):
                        ncol = cnb * P
                        c0 = cb * P
                        xc = bpool.tile([HID, ncol], f16, tag="xc")
                        nc.sync.dma_start(out=xc[:], in_=xpT[:, c0:c0 + ncol])
                        pm = bpsum.tile([HID, ncol], f32, tag="bp1")
                        nc.tensor.matmul(pm[:], wsb["w1"][:], xc[:], start=True, stop=True)
                        hh1 = bpool.tile([HID, ncol], f16, tag="hh1")
                        nc.scalar.activation(hh1[:], pm[:], AF.Relu, bias=bsb["b1"][:, :])
                        pm2 = bpsum.tile([HID, ncol], f32, tag="bp2")
                        nc.tensor.matmul(pm2[:], wsb["w2"][:], hh1[:], start=True, stop=True)
                        hh2 = bpool.tile([HID, ncol], f16, tag="hh2")
                        nc.scalar.activation(hh2[:], pm2[:], AF.Relu, bias=bsb["b2"][:, :])
                        pm3 = bpsum.tile([HID, ncol], f32, tag="bp3")
                        nc.tensor.matmul(pm3[:], wsb["w3"][:], hh2[:], start=True, stop=True)
                        hTc = bpool.tile([HID, ncol], f16, tag="hTc")
                        nc.scalar.activation(hTc[:], pm3[:], AF.Identity, bias=bsb["b3"][:, :])
                        nc.sync.dma_start(out=hT_d[:, c0:c0 + ncol], in_=hTc[:])
                        # transpose to dest-major: in [64, 128] -> out [128, 64]
                        for b in range(cnb):
                            tp = bpsum.tile([P, HID], f16, tag="btp")
                            nc.tensor.transpose(tp[:], hTc[:, b * P:(b + 1) * P],
                                                ident16[0:HID, 0:HID])
                            nc.scalar.copy(out=h_dm[:, cb + b, :], in_=tp[:])
                        hd = h_dm[:, cb:cb + cnb, :]
                        sq = bpool.tile([P, cnb, HID], f32, tag="sq")
                        nc.scalar.activation(sq[:], hd, AF.Square)
                        n2 = bpool.tile([P, cnb], f32, tag="n2")
                        nc.vector.tensor_reduce(out=n2[:], in_=sq[:], axis=AX.X, op=OP.add)
                        nrm = bpool.tile([P, cnb], f32, tag="nrm")
                        nc.scalar.activation(nrm[:], n2[:], AF.Sqrt)
                        nrme = bpool.tile([P, cnb], f32, tag="nrme")
                        nc.scalar.activation(nrme[:], nrm[:], AF.Identity, bias=epsb[:, :])
                        rn = bpool.tile([P, cnb], f32, tag="rn")
                        nc.vector.reciprocal(rn[:], nrme[:])
                        nx = bpool.tile([P, cnb], f32, tag="nx")
                        nc.vector.tensor_tensor(out=nx[:], in0=nrm[:], in1=rn[:], op=OP.mult)
                        nc.scalar.activation(nx2_sb[:, cb:cb + cnb], nx[:], AF.Square)
                        rnm = bpool.tile([P, cnb], f32, tag="rnm")
                        nc.vector.tensor_tensor(out=rnm[:], in0=rn[:],
                                                in1=mask_sb[:, cb:cb + cnb], op=OP.mult)
                        st = bpool.tile([P, cnb, CH], f16, tag="st")
                        nc.vector.tensor_tensor(
                            out=st[:, :, 0:HID], in0=hd,
                            in1=dinv_sb[:, cb:cb + cnb].unsqueeze(2).to_broadcast([P, cnb, HID]),
                            op=OP.mult)
                        nc.vector.tensor_tensor(
                            out=st[:, :, HID:CH], in0=hd,
                            in1=rnm[:].unsqueeze(2).to_broadcast([P, cnb, HID]),
                            op=OP.mult)
                        nc.sync.dma_start(
                            out=shard[:, :].rearrange("(p b) c -> p b c", b=NBUCK)[:, cb:cb + cnb, :],
                            in_=st[:])

                # ---- all-gather the table (dummy rows are exact zeros) ----
                for _rc in range(KR_COLL):
                    nc.gpsimd.collective_compute(
                        "AllGather", mybir.AluOpType.bypass,
                        replica_groups=[list(range(NCORE))],
                        ins=[shard[:, :]], outs=[table[:, :]])

                if KDBG == "build":
                    with tc.tile_pool(name="dbgp", bufs=1) as dp:
                        dt_ = dp.tile([P, CH], f16)
                        nc.sync.dma_start(out=dt_[:], in_=table[50000:50000 + P, :])
                        dt32 = dp.tile([P, CH], f32)
                        nc.scalar.copy(out=dt32[:], in_=dt_[:])
                        nc.sync.dma_start(out=dbg[:], in_=dt32[:])
                # ---- edge phase ----
                with tc.tile_pool(name="io", bufs=4) as io, \
                     tc.tile_pool(name="edge", bufs=3) as ep:
                  for _re in range(KR_EDGE):
                    g16 = x16 = soff = 0
                    nsub = 0
                    qrr = [0]
                    for w in (range(NWIN) if KDBG != "build" else []):
                        base = w * WIN
                        bend = min((w + 1) * WIN, TABLE_ROWS)
                        for sub in structure[w]:
                            nb, K = sub["nb"], sub["K"]
                            S = nb * K
                            nsub += 1
                            if nsub > KEDGE:
                                g16 += S * 8
                                x16 += nb * 8
                                soff += S
                                continue
                            it = io.tile([P, S * 8], i16, tag="it")
                            nc.sync.dma_start(out=it[:], in_=gidx[:, g16:g16 + S * 8])
                            xt = io.tile([P, nb * 8], i16, tag="xt")
                            nc.sync.dma_start(out=xt[:], in_=xidx[:, x16:x16 + nb * 8])
                            stx = io.tile([P, nb * 8], i16, tag="stx")
                            nc.sync.dma_start(out=stx[:], in_=sidx[:, x16:x16 + nb * 8])
                            sd = io.tile([P, S], f16, tag="sd")
                            nc.sync.dma_start(out=sd[:], in_=sdegt[:, soff:soff + S])
                            g16 += S * 8
                            x16 += nb * 8
                            soff += S

                            gt = ep.tile([P, S, CH], f16, tag="gt")
                            if "gt" in KSKIP:
                                nc.vector.memset(gt[:, :, :], 0.0)
                            else:
                                for j0 in range(0, S, GCHUNK):
                                    ns = min(GCHUNK, S - j0)
                                    nc.gpsimd.dma_gather(
                                        out_ap=gt[:, j0:j0 + ns, :],
                                        in_ap=table[base:bend, :],
                                        idxs_ap=it[:, j0 * 8:(j0 + ns) * 8],
                                        num_idxs=ns * P, num_idxs_reg=ns * P,
                                        elem_size=CH, queue_num=qrr[0] % 4)
                                    qrr[0] += 1
                            xc2 = ep.tile([P, nb, CH], f16, tag="xc2")
                            if "xc" in KSKIP:
                                nc.vector.memset(xc2[:, :, :], 0.0)
                            else:
                                nc.gpsimd.dma_gather(
                                    out_ap=xc2[:, :, :], in_ap=shard[:, :],
                                    idxs_ap=xt[:, :], num_idxs=nb * P, num_idxs_reg=nb * P,
                                    elem_size=CH, queue_num=qrr[0] % 4)
                                qrr[0] += 1

                            gat = gt[:, :, :].rearrange("p (b k) c -> p b k c", b=nb)
                            tmpa = ep.tile([P, S, HID], f16, tag="tmpa")
                            nc.vector.tensor_tensor(
                                out=tmpa[:, :, :].rearrange("p (b k) c -> p b k c", b=nb),
                                in0=gat[:, :, :, HID:CH],
                                in1=xc2[:, :, HID:CH].unsqueeze(2).to_broadcast([P, nb, K, HID]),
                                op=OP.mult)
                            alpha = ep.tile([P, S], f16, tag="alpha")
                            with nc.allow_low_precision("cosine logits bounded by 1"):
                                nc.vector.tensor_reduce(out=alpha[:], in_=tmpa[:, :, :],
                                                        axis=AX.X, op=OP.add)
                            e = ep.tile([P, S], f16, tag="e")
                            nc.scalar.activation(e[:], alpha[:], AF.Exp, scale=beta128[:, :])
                            epw = ep.tile([P, S], f16, tag="epw")
                            nc.vector.tensor_tensor(out=epw[:], in0=e[:], in1=sd[:], op=OP.mult)
                            tmpn = ep.tile([P, S, HID], f16, tag="tmpn")
                            nc.vector.tensor_tensor(
                                out=tmpn[:, :, :], in0=gt[:, :, 0:HID],
                                in1=epw[:].unsqueeze(2).to_broadcast([P, S, HID]),
                                op=OP.mult)
                            part = ep.tile([P, nb, ACC_W], f32, tag="part")
                            nc.vector.memset(part[:, :, CH + 1:ACC_W], 0.0)
                            nc.vector.tensor_reduce(
                                out=part[:, :, CH:CH + 1],
                                in_=e[:].rearrange("p (b k) -> p b k", b=nb),
                                axis=AX.X, op=OP.add)
                            # segment-sum gat hn / tmpn over k by in-place
                            # contiguous halving adds (strided tensor_reduce
                            # runs at 1 elem/cycle; this stays in 16-bit 2x)
                            tmn = tmpn[:, :, :].rearrange("p (b k) c -> p b k c", b=nb)
                            for view, dst in ((gat, part[:, :, 0:HID]),
                                              (tmn, part[:, :, HID:CH])):
                                k = K
                                while k > 2:
                                    h = (k + 1) // 2
                                    nc.vector.tensor_tensor(
                                        out=view[:, :, 0:k - h, 0:HID],
                                        in0=view[:, :, 0:k - h, 0:HID],
                                        in1=view[:, :, h:k, 0:HID], op=OP.add)
                                    k = h
                                if k == 2:
                                    nc.vector.tensor_tensor(
                                        out=dst, in0=view[:, :, 0, 0:HID],
                                        in1=view[:, :, 1, 0:HID], op=OP.add)
                                else:
                                    nc.vector.tensor_scalar_add(
                                        dst, view[:, :, 0, 0:HID], 0.0)
                            if "sc" not in KSKIP:
                                nc.gpsimd.dma_scatter_add(
                                    out_ap=accum[:, :], in_ap=part[:, :, :], idxs_ap=stx[:, :],
                                    num_idxs=nb * P, num_idxs_reg=nb * P, elem_size=ACC_W,
                                    queue_num=qrr[0] % 4)
                                qrr[0] += 1

                if KDBG == "edge":
                    with tc.tile_pool(name="dbgp2", bufs=1) as dp2:
                        for r0 in range(0, ACC_ROWS, P):
                            r1 = min(r0 + P, ACC_ROWS)
                            da = dp2.tile([P, ACC_W], f32, tag="da")
                            nc.sync.dma_start(out=da[:r1 - r0, :], in_=accum[r0:r1, :])
                            nc.sync.dma_start(out=dbga[r0:r1, :], in_=da[:r1 - r0, :])
                # ---- epilogue ----
                with tc.tile_pool(name="epi", bufs=2) as epi, \
                     tc.tile_pool(name="epsum", bufs=3, space="PSUM") as epsum, \
                     tc.tile_pool(name="epsum2", bufs=2, space="PSUM") as epsum2:
                  for _rp in range(KR_EPI):
                    for ci, (cb, cnb) in enumerate(chunks if KDBG not in ("build", "edge") else []):
                        ncol = cnb * P
                        c0 = cb * P
                        acc = epi.tile([P, cnb, ACC_W], f32, tag="acc")
                        nc.sync.dma_start(
                            out=acc[:],
                            in_=accum[0:DCORE, :].rearrange("(p b) c -> p b c", b=NBUCK)[:, cb:cb + cnb, :])
                        es = epi.tile([P, cnb], f32, tag="es")
                        nc.scalar.activation(es[:], nx2_sb[:, cb:cb + cnb], AF.Exp,
                                             scale=beta128[:, :])
                        denf = epi.tile([P, cnb], f32, tag="denf")
                        nc.vector.tensor_tensor(out=denf[:], in0=acc[:, :, CH:CH + 1].squeeze(2),
                                                in1=padn_sb[:, cb:cb + cnb], op=OP.add)
                        nc.vector.tensor_tensor(out=denf[:], in0=denf[:], in1=es[:], op=OP.add)
                        rec = epi.tile([P, cnb], f32, tag="rec")
                        nc.vector.reciprocal(rec[:], denf[:])
                        hd = h_dm[:, cb:cb + cnb, :]
                        numf = epi.tile([P, cnb, HID], f32, tag="numf")
                        nc.vector.tensor_tensor(
                            out=numf[:], in0=hd,
                            in1=es[:].unsqueeze(2).to_broadcast([P, cnb, HID]), op=OP.mult)
                        nc.vector.tensor_tensor(out=numf[:], in0=numf[:],
                                                in1=acc[:, :, HID:CH], op=OP.add)
                        h1 = epi.tile([P, cnb, HID], f16, tag="h1")
                        nc.vector.tensor_tensor(
                            out=h1[:], in0=numf[:],
                            in1=rec[:].unsqueeze(2).to_broadcast([P, cnb, HID]), op=OP.mult)
                        ag2f = epi.tile([P, cnb, HID], f32, tag="ag2f")
                        dv = dinv_sb[:, cb:cb + cnb].unsqueeze(2).to_broadcast([P, cnb, HID])
                        nc.vector.tensor_tensor(out=ag2f[:], in0=hd, in1=dv, op=OP.mult)
                        nc.vector.tensor_tensor(out=ag2f[:], in0=ag2f[:],
                                                in1=acc[:, :, 0:HID], op=OP.add)
                        agg2 = epi.tile([P, cnb, HID], f16, tag="agg2")
                        nc.vector.tensor_tensor(out=agg2[:], in0=ag2f[:], in1=dv, op=OP.mult)
                        aggT = epi.tile([HID, ncol], f16, tag="aggT")
                        h1T = epi.tile([HID, ncol], f16, tag="h1T")
                        for b in range(cnb):
                            tp1 = epsum2.tile([HID, P], f16, tag="etp")
                            nc.tensor.transpose(tp1[:], agg2[:, b, :], ident16[:, :])
                            nc.scalar.copy(out=aggT[:, b * P:(b + 1) * P], in_=tp1[:])
                            tp2 = epsum2.tile([HID, P], f16, tag="etp")
                            nc.tensor.transpose(tp2[:], h1[:, b, :], ident16[:, :])
                            nc.scalar.copy(out=h1T[:, b * P:(b + 1) * P], in_=tp2[:])
                        pf0 = epsum.tile([HID, ncol], f32, tag="mm")
                        nc.tensor.matmul(pf0[:], wsb["wg1"][:], aggT[:], start=True, stop=True)
                        f0T = epi.tile([HID, ncol], f16, tag="f0T")
                        nc.vector.tensor_scalar_add(f0T[:], pf0[:], bsb["bg1"][:, :])
                        pf1 = epsum.tile([HID, ncol], f32, tag="mm")
                        nc.tensor.matmul(pf1[:], wsb["wg2"][:], aggT[:], start=True, stop=True)
                        f1T = epi.tile([HID, ncol], f16, tag="f1T")
                        nc.vector.tensor_scalar_add(f1T[:], pf1[:], bsb["bg2"][:, :])
                        pp0 = epsum.tile([HID, ncol], f32, tag="mm")
                        nc.tensor.matmul(pp0[:], wsb["wf"][:], f0T[:], start=True, stop=True)
                        p0T = epi.tile([HID, ncol], f16, tag="p0T")
                        nc.scalar.activation(p0T[:], pp0[:], AF.Tanh, bias=bsb["bf"][:, :])
                        pp1 = epsum.tile([HID, ncol], f32, tag="mm")
                        nc.tensor.matmul(pp1[:], wsb["wf"][:], f1T[:], start=True, stop=True)
                        p1T = epi.tile([HID, ncol], f16, tag="p1T")
                        nc.scalar.activation(p1T[:], pp1[:], AF.Tanh, bias=bsb["bf"][:, :])
                        hTl = epi.tile([HID, ncol], f16, tag="hTl")
                        nc.sync.dma_start(out=hTl[:], in_=hT_d[:, c0:c0 + ncol])
                        ppx = epsum.tile([HID, ncol], f32, tag="mm")
                        nc.tensor.matmul(ppx[:], wsb["wx"][:], hTl[:], start=True, stop=True)
                        xpj = epi.tile([HID, ncol], f16, tag="xpj")
                        nc.scalar.activation(xpj[:], ppx[:], AF.Tanh, bias=bsb["bx"][:, :])
                        t0 = epi.tile([HID, ncol], f16, tag="t0")
                        nc.vector.tensor_tensor(out=t0[:], in0=p0T[:], in1=xpj[:], op=OP.mult)
                        t1 = epi.tile([HID, ncol], f16, tag="t1")
                        nc.vector.scalar_tensor_tensor(
                            out=t1[:], in0=p1T[:], scalar=-1.0, in1=xpj[:],
                            op0=OP.mult, op1=OP.mult)
                        pl = epsum2.tile([1, ncol], f32, tag="psmall")
                        nc.tensor.matmul(pl[:], ones_col[:], t0[:], start=True, stop=False)
                        nc.tensor.matmul(pl[:], ones_col[:], t1[:], start=False, stop=True)
                        s0 = epi.tile([1, ncol], f16, tag="s0")
                        nc.scalar.activation(s0[:], pl[:], AF.Sigmoid)
                        ps0 = epsum.tile([HID, ncol], f32, tag="mm")
                        nc.tensor.matmul(ps0[:], ones_row[:], s0[:], start=True, stop=True)
                        d01 = epi.tile([HID, ncol], f16, tag="d01")
                        nc.vector.tensor_tensor(out=d01[:], in0=f0T[:], in1=f1T[:],
                                                op=OP.subtract)
                        nc.vector.tensor_tensor(out=d01[:], in0=d01[:], in1=ps0[:], op=OP.mult)
                        resT = epi.tile([HID, ncol], f16, tag="resT")
                        nc.vector.tensor_tensor(out=resT[:], in0=d01[:], in1=f1T[:], op=OP.add)
                        py = epsum2.tile([2, ncol], f32, tag="psmall")
                        nc.tensor.matmul(py[:], wc0_sb[:], resT[:], start=True, stop=False)
                        nc.tensor.matmul(py[:], wc1_sb[:], h1T[:], start=False, stop=True)
                        ysb = epi.tile([2, ncol], f32, tag="ysb")
                        nc.scalar.activation(ysb[:], py[:], AF.Identity, bias=bc_sb[:, :])
                        nc.sync.dma_start(out=out[:, c0:c0 + ncol], in_=ysb[:])

    nc.compile()
    return nc


_CACHE = {}


def kernel(**inputs):
    from concourse.bass_utils import run_bass_kernel_spmd

    x = np.asarray(inputs["x"], np.float32)
    edge_index = np.asarray(inputs["edge_index"])
    cores, meta = _prep(x, edge_index)
    structure = meta["structure"]
    gw = cores[0]["gidx"].shape[1]
    xw = cores[0]["xidx"].shape[1]
    sw = cores[0]["sdeg"].shape[1]

    key = (gw, xw, sw, tuple(meta["ZROW"]),
           tuple((s["b0"], s["nb"], s["K"]) for w in structure for s in w))
    if key not in _CACHE:
        _CACHE[key] = _build(structure, meta["ZROW"], gw, xw, sw)
    nc = _CACHE[key]

    shared = {}
    for n in ("w1", "w2", "w3", "wg1", "wg2", "wf", "wx"):
        shared[n] = np.ascontiguousarray(np.asarray(inputs[n], np.float16))
    wc = np.asarray(inputs["wc"], np.float32)
    shared["wc0"] = np.ascontiguousarray(wc[0:HID, :]).astype(np.float16)
    shared["wc1"] = np.ascontiguousarray(wc[HID:2 * HID, :]).astype(np.float16)
    for n in ("b1", "b2", "b3", "bg1", "bg2", "bf", "bx"):
        shared[n] = np.asarray(inputs[n], np.float32).reshape(HID, 1)
    shared["bc"] = np.asarray(inputs["bc"], np.float32).reshape(2, 1)
    shared["beta"] = np.asarray(inputs["beta"], np.float32).reshape(1, 1)

    in_maps = []
    for k in range(NCORE):
        m = dict(shared)
        m["xpT"] = cores[k]["xpT"]
        m["gidx"] = cores[k]["gidx"]
        m["xidx"] = cores[k]["xidx"]
        m["sidx"] = cores[k]["sidx"]
        m["sdegt"] = cores[k]["sdeg"]
        m["dinvt"] = cores[k]["dinvc"]
        m["maskt"] = cores[k]["maskc"]
        m["padnt"] = cores[k]["padneg"]
        in_maps.append(m)

    res = run_bass_kernel_spmd(nc, in_maps, core_ids=list(range(NCORE)))
    _last_run["nc"] = nc
    _last_run["in_maps"] = in_maps

    y = np.zeros((N, 2), np.float32)
    for k in range(NCORE):
        sn = meta["slot_node"][k * DCORE:(k + 1) * DCORE]
        sel = sn >= 0
        y[sn[sel]] = res.results[k]["out"].T[sel]
    return y


# exposed for test harness timing
_last_run = {}
